# revision 12
# baseline (speedup 1.0000x reference)
"""Trainium2 Bass kernel for nn_MultiHeadAttention (B=4, S=2048, D=1024, H=16).

Sharding: 8 cores = batch(4) x head-half(2).  Each core computes, for its
batch element, 8 of the 16 heads: QKV projections against column-sliced
weights, causal attention, and the output projection against the matching
row-slice of Wo.  The two bf16 partial outputs per batch element are summed
on the host (replaces the tensor-parallel all-reduce), and Wo_b is added
there.

v4 design notes:
- Attention scores use the transposed layout scoresT[k, q]; the softmax
  denominator comes from an all-ones column prepended to each head's V
  block (so it lands on psum partition 0, where the fast reciprocal can
  read it straight out of PSUM); normalization runs off the critical path.
- The K=64 scores matmuls are row-packed in the PE array: even kb targets
  array rows 0-63, odd kb rows 64-127 (explicit tile_position), using
  half-swapped copies qT2/kT2 so the operands sit on the matching SBUF
  partitions.  Adjacent even/odd scores matmuls run concurrently -> ~2x
  scores throughput.
- exp on the Scalar engine is the attention-phase serial bottleneck
  (~166us).  The PE stream is kept stall-free (idle gaps re-throttle the
  PE clock to 1.2 GHz): scores for kb-pair g+1 are emitted one group ahead
  of the exps of pair g, and projection / output-projection matmul groups
  are paced into the attention stream by a cycle ledger so the PE always
  has dependency-free work while ACT churns through the exps.
"""

import sys

if "/opt/trn_rl_repo" not in sys.path:
    sys.path.insert(0, "/opt/trn_rl_repo")

import numpy as np
import ml_dtypes

B, S, D = 4, 2048, 1024
H, HD = 16, 64
HH = H // 2          # heads per core
DH = D // 2          # local attention feature dim (HH * HD)
N_CORES = 8
QH = 1024            # q-range processed per attention pass (psum budget)

DT_MODE = "bf16"

_CACHE = {}


def _build(dt_mode):
    import concourse.bass as bass
    import concourse.mybir as mybir
    from concourse import bacc
    from concourse.tile import TileContext
    from concourse.masks import make_upper_triangular

    F32 = mybir.dt.float32
    DT = mybir.dt.bfloat16 if dt_mode == "bf16" else mybir.dt.float32

    ADD = mybir.AluOpType.add
    MULT = mybir.AluOpType.mult
    EXP = mybir.ActivationFunctionType.Exp

    nc = bacc.Bacc("TRN2", target_bir_lowering=False, debug=False,
                   num_devices=N_CORES)

    xT = nc.dram_tensor("xT", [D, S], DT, kind="ExternalInput").ap()
    wq = nc.dram_tensor("wq", [D, DH], DT, kind="ExternalInput").ap()
    wk = nc.dram_tensor("wk", [D, DH], DT, kind="ExternalInput").ap()
    wv = nc.dram_tensor("wv", [D, DH], DT, kind="ExternalInput").ap()
    wo = nc.dram_tensor("wo", [DH, D], DT, kind="ExternalInput").ap()
    bq = nc.dram_tensor("bq", [128, DH // 128], F32, kind="ExternalInput").ap()
    bk = nc.dram_tensor("bk", [128, DH // 128], F32, kind="ExternalInput").ap()
    bv = nc.dram_tensor("bv", [128, DH], F32, kind="ExternalInput").ap()
    out = nc.dram_tensor("out", [S, D], DT, kind="ExternalOutput").ap()

    ND = D // 128        # 8 contraction tiles over D
    NS = S // 128        # 16 s-blocks
    NJ = DH // 128       # 4 feature tiles of the local 512 dim
    NSC = S // 512       # 4 columns of 512 over S

    from contextlib import ExitStack

    with TileContext(nc) as tc:
        with ExitStack() as stack:
            pool = lambda *a, **kw: stack.enter_context(tc.tile_pool(*a, **kw))
            pp = pool(name="persist", bufs=1)
            pqT = pool(name="qT", bufs=NJ)
            pkT = pool(name="kT", bufs=NJ)
            pqT2 = pool(name="qT2", bufs=NJ)
            pkT2 = pool(name="kT2", bufs=NJ)
            pv = pool(name="vaug", bufs=NS)
            pattnT = pool(name="attnT", bufs=NJ)
            pxt = pool(name="xt", bufs=ND)
            pw = pool(name="w", bufs=3 * ND)
            pwo = pool(name="wo", bufs=NJ)
            pproj = pool(name="projps", bufs=2, space="PSUM")
            pexp = pool(name="exp", bufs=4)
            pau = pool(name="au", bufs=2)
            pof = pool(name="of", bufs=2)
            prc = pool(name="recip", bufs=2)
            pscps = pool(name="scps", bufs=2, space="PSUM")
            patps = pool(name="atps", bufs=1, space="PSUM")
            # ---- constants / biases ----
            bq_t = pp.tile([128, NJ], F32, tag="bq")
            nc.sync.dma_start(bq_t[:], bq[:])
            bk_t = pp.tile([128, NJ], F32, tag="bk")
            nc.sync.dma_start(bk_t[:], bk[:])
            bv_t = pp.tile([128, DH], F32, tag="bv")
            nc.sync.dma_start(bv_t[:], bv[:])
            ones_t = pp.tile([128, HH], F32, tag="ones")
            nc.gpsimd.memset(ones_t[:], 1.0)
            # causal mask for diagonal 128x128 squares of scoresT[k, q]:
            # valid (k <= q) <=> partition p <= free f -> upper-tri incl diag.
            mask_f = pp.tile([128, 128], F32, tag="maskf")
            make_upper_triangular(nc, mask_f[:], val=1.0, diag=True)
            mask_t = pp.tile([128, 128], DT, tag="mask")
            nc.vector.tensor_copy(mask_t[:], mask_f[:])

            # persistent activation buffers
            qT_t = [pqT.tile([128, S], DT, tag="qT", name=f"qT{i}")
                    for i in range(NJ)]
            kT_t = [pkT.tile([128, S], DT, tag="kT", name=f"kT{i}")
                    for i in range(NJ)]
            qT2_t = [pqT2.tile([128, S], DT, tag="qT2", name=f"qT2{i}")
                     for i in range(NJ)]
            kT2_t = [pkT2.tile([128, S], DT, tag="kT2", name=f"kT2{i}")
                     for i in range(NJ)]
            v_t = [pv.tile([128, HH * (HD + 1)], DT, tag="vaug",
                           name=f"vaug{i}") for i in range(NS)]
            aT_t = [pattnT.tile([128, S], DT, tag="attnT", name=f"attnT{i}")
                    for i in range(NJ)]

            # input DMAs, ordered so the first V matmuls start early
            xt_t = [pxt.tile([128, S], DT, tag="xt", name=f"xt{i}")
                    for i in range(ND)]
            wv_t = [pw.tile([128, DH], DT, tag="w3", name=f"wv{db}")
                    for db in range(ND)]
            wq_t = [pw.tile([128, DH], DT, tag="w3", name=f"wq{db}")
                    for db in range(ND)]
            wk_t = [pw.tile([128, DH], DT, tag="w3", name=f"wk{db}")
                    for db in range(ND)]
            wo_t = [pwo.tile([128, D], DT, tag="wo", name=f"wo{db}")
                    for db in range(NJ)]

            def dma_w(w_t, ap):
                for db in range(ND):
                    nc.sync.dma_start(w_t[db][:], ap[db * 128:(db + 1) * 128, :])

            def dma_x(cc):
                cs = slice(cc * 512, (cc + 1) * 512)
                for db in range(ND):
                    nc.sync.dma_start(xt_t[db][:, cs],
                                      xT[db * 128:(db + 1) * 128, cs])

            dma_w(wv_t, wv)
            dma_x(0)
            dma_x(1)
            dma_w(wq_t, wq)
            dma_x(2)
            dma_x(3)
            dma_w(wk_t, wk)
            for db in range(NJ):
                nc.sync.dma_start(wo_t[db][:], wo[db * 128:(db + 1) * 128, :])

            # ---- work-unit emitters (each ~850ns of PE work) ----
            live_ps = {}

            def v_half(sb, half):
                name = f"psv{sb}"
                if half == 0:
                    ps = pproj.tile([128, 512], F32, tag="proj", name=name)
                    live_ps[name] = ps
                else:
                    ps = live_ps.pop(name)
                for db in range(4 * half, 4 * half + 4):
                    nc.tensor.matmul(
                        ps[:],
                        lhsT=xt_t[db][:, sb * 128:(sb + 1) * 128],
                        rhs=wv_t[db][:],
                        start=(db == 0), stop=(db == ND - 1),
                    )
                if half == 1:
                    # v_aug layout per head: [v+bias | ones] (65 cols); the
                    # ones column makes psum row 64 the softmax denominator.
                    vt = v_t[sb]
                    v3 = vt[:].rearrange("p (h e) -> p h e", e=HD + 1)
                    nc.vector.tensor_tensor(
                        v3[:, :, 0:HD],
                        ps[:].rearrange("p (h e) -> p h e", e=HD),
                        bv_t[:].rearrange("p (h e) -> p h e", e=HD),
                        op=ADD,
                    )
                    nc.vector.tensor_copy(
                        v3[:, :, HD:HD + 1],
                        ones_t[:].rearrange("p (h e) -> p h e", e=1),
                    )

            def qk_half(w_t, bias_t, dstT, jb, sc, half):
                name = f"ps{'q' if dstT is qT_t else 'k'}{jb}_{sc}"
                if half == 0:
                    ps = pproj.tile([128, 512], F32, tag="proj", name=name)
                    live_ps[name] = ps
                else:
                    ps = live_ps.pop(name)
                for db in range(4 * half, 4 * half + 4):
                    nc.tensor.matmul(
                        ps[:],
                        lhsT=w_t[db][:, jb * 128:(jb + 1) * 128],
                        rhs=xt_t[db][:, sc * 512:(sc + 1) * 512],
                        start=(db == 0), stop=(db == ND - 1),
                    )
                if half == 1:
                    nc.vector.tensor_scalar_add(
                        dstT[jb][:, sc * 512:(sc + 1) * 512],
                        ps[:], bias_t[:, jb:jb + 1],
                    )

            def swap_copy(jb):
                # half-swapped copies so odd-kb scores matmuls can target
                # the other PE row group
                for src, dst in ((qT_t[jb], qT2_t[jb]), (kT_t[jb], kT2_t[jb])):
                    nc.vector.tensor_copy(dst[0:64, :], src[64:128, :])
                    nc.vector.tensor_copy(dst[64:128, :], src[0:64, :])

            def o_group(sb, jc):
                ps = pproj.tile([128, 512], F32, tag="proj",
                                name=f"pso{sb}_{jc}")
                for db in range(NJ):
                    nc.tensor.matmul(
                        ps[:],
                        lhsT=aT_t[db][:, sb * 128:(sb + 1) * 128],
                        rhs=wo_t[db][:, jc * 512:(jc + 1) * 512],
                        start=(db == 0), stop=(db == NJ - 1),
                    )
                ot = pof.tile([128, 512], DT, tag="of", name=f"ot{sb}_{jc}")
                nc.vector.tensor_copy(ot[:], ps[:])
                nc.sync.dma_start(
                    out[sb * 128:(sb + 1) * 128, jc * 512:(jc + 1) * 512],
                    ot[:],
                )

            # filler queue: (req_key, min_gidx, pe_ns, closure).
            # req_key = (gidx, g) by which the unit MUST have been emitted
            # (dependency order); gidx = 2*h + qh, g = kb-pair index within
            # that (h, qh).  min_gidx: earliest gidx at which the unit MAY
            # run (outproj needs all heads' qh=0 rows written first).
            filler = []
            for sb in range(8, NS):          # v for kb 8-15: jit by h0/qh1
                for half in range(2):
                    filler.append(((1, sb // 2), 0, 853,
                                   lambda s=sb, hf=half: v_half(s, hf)))
            for jb in range(1, NJ):          # qk for jb: by h=2*jb
                for sc in range(NSC):
                    for w_t, bias_t, dstT in ((wq_t, bq_t, qT_t),
                                              (wk_t, bk_t, kT_t)):
                        for half in range(2):
                            filler.append(
                                ((4 * jb, 0), 0, 853,
                                 lambda w=w_t, b=bias_t, d=dstT, j=jb,
                                        s=sc, hf=half:
                                    qk_half(w, b, d, j, s, hf)))
                filler.append(((4 * jb, 0), 0, 0, lambda j=jb: swap_copy(j)))
            for sb in range(8):              # outproj rows 0-1023: in h7/qh1
                for jc in range(2):
                    filler.append(((99, 0), 15, 853,
                                   lambda s=sb, j=jc: o_group(s, j)))

            debt = [0.0]

            def emit_filler_by_debt(gidx):
                while debt[0] > 900 and filler and filler[0][1] <= gidx:
                    unit = filler.pop(0)
                    unit[3]()
                    debt[0] -= unit[2]

            def force_filler(key):
                while filler and filler[0][0] <= key:
                    unit = filler.pop(0)
                    unit[3]()
                    debt[0] -= unit[2]

            # ---- phase A: V(sb 0-7) + QK(jb=0) ----
            for sb in range(8):
                v_half(sb, 0)
                v_half(sb, 1)
            for sc in range(NSC):
                for w_t, bias_t, dstT in ((wq_t, bq_t, qT_t),
                                          (wk_t, bk_t, kT_t)):
                    qk_half(w_t, bias_t, dstT, 0, sc, 0)
                    qk_half(w_t, bias_t, dstT, 0, sc, 1)
            swap_copy(0)

            # ---- phase B: attention ----
            def chunk_cols(lo):
                chunks = []
                c = lo
                while c < QH:
                    c1 = min((c // 512 + 1) * 512, QH)
                    chunks.append((c, c1))
                    c = c1
                return chunks

            for h in range(HH):
                hb, hr = h // 2, (h % 2) * 64
                hr2 = 64 - hr
                vcol = h * (HD + 1)
                for qh in range(S // QH):
                    gidx = 2 * h + qh
                    q0 = qh * QH
                    at = patps.tile([65, QH], F32, tag="at",
                                    name=f"at{h}_{qh}")
                    nkb = (q0 + QH) // 128

                    def scores(kb):
                        k0 = kb * 128
                        lo = max(k0 - q0, 0)
                        sc = pscps.tile([128, QH], F32, tag="sc",
                                        name=f"sc{h}_{qh}_{kb}")
                        if kb % 2 == 0:
                            kT, qT, rp = kT_t[hb], qT_t[hb], hr
                        else:
                            kT, qT, rp = kT2_t[hb], qT2_t[hb], hr2
                        for (c0, c1) in chunk_cols(lo):
                            nc.tensor.matmul(
                                sc[:, c0:c1],
                                lhsT=kT[rp:rp + 64, k0:k0 + 128],
                                rhs=qT[rp:rp + 64, q0 + c0:q0 + c1],
                                start=True, stop=True,
                                tile_position=(rp, 0),
                            )
                        return sc

                    def exp(kb, sc):
                        k0 = kb * 128
                        lo = max(k0 - q0, 0)
                        et = pexp.tile([128, QH], DT, tag="exp",
                                       name=f"et{h}_{qh}_{kb}")
                        nc.scalar.activation(et[:, lo:QH], sc[:, lo:QH],
                                             EXP, scale=1.0 / np.sqrt(HD))
                        if k0 >= q0:
                            nc.vector.tensor_mul(et[:, lo:lo + 128],
                                                 et[:, lo:lo + 128],
                                                 mask_t[:])
                        return et

                    def pv(kb, et):
                        lo = max(kb * 128 - q0, 0)
                        for (c0, c1) in chunk_cols(lo):
                            nc.tensor.matmul(
                                at[0:65, c0:c1],
                                lhsT=v_t[kb][:, vcol:vcol + HD + 1],
                                rhs=et[:, c0:c1],
                                start=(kb == 0),
                                stop=(kb == (q0 + c1 - 1) // 128),
                            )

                    def est(kb):
                        lo = max(kb * 128 - q0, 0)
                        n = QH - lo
                        return (n + 352) / 1.2, n / 2.4

                    # Emission order per pair-group g.  The PE queue is
                    # in-order, so anything emitted ahead of ready work
                    # head-of-line-blocks it; the PVs therefore LAG one
                    # group (their exps finished last group), filler comes
                    # next (dependency-free), and the lookahead scores pair
                    # goes last (its sc-slot WAR on this group's exps has
                    # cleared by the time the PE drains the earlier work).
                    #   ACT: exp(2g), exp(2g+1)
                    #   PE : pv(2g-2), pv(2g-1) | filler | scores(2g+2/3)
                    npair = nkb // 2
                    force_filler((gidx, 0))
                    sc_pair = [scores(0), scores(1)]
                    prev_ets = None
                    for g in range(npair):
                        et0 = exp(2 * g, sc_pair[0])
                        et1 = exp(2 * g + 1, sc_pair[1])
                        if prev_ets is not None:
                            pv(2 * g - 2, prev_ets[0])
                            pv(2 * g - 1, prev_ets[1])
                        a0, p0 = est(2 * g)
                        a1, p1 = est(2 * g + 1)
                        # scores pair is concurrent (~p0), PVs are serial
                        debt[0] += (a0 + a1) - (p0 + p0 + p1)
                        emit_filler_by_debt(gidx)
                        if g + 1 < npair:
                            force_filler((gidx, g + 1))
                            sc_pair = [scores(2 * g + 2), scores(2 * g + 3)]
                        prev_ets = (et0, et1)
                    pv(nkb - 2, prev_ets[0])
                    pv(nkb - 1, prev_ets[1])

                    # drain + normalize off the critical path (PSUM/SBUF
                    # partition bases must be 32-aligned, so two copies).
                    au = pau.tile([64, QH], F32, tag="au", name=f"au{h}_{qh}")
                    nc.vector.tensor_copy(au[:], at[0:64, :])
                    dn = prc.tile([1, QH], F32, tag="dn", name=f"dn{h}_{qh}")
                    nc.vector.tensor_copy(dn[:], at[64:65, :])
                    rc = prc.tile([1, QH], F32, tag="rc", name=f"rc{h}_{qh}")
                    nc.vector.reciprocal_approx_fast(rc[:], dn[:])
                    bc = prc.tile([64, QH], F32, tag="bc", name=f"bc{h}_{qh}")
                    nc.gpsimd.partition_broadcast(bc[:], rc[:])
                    nc.gpsimd.tensor_tensor(
                        aT_t[hb][hr:hr + 64, q0:q0 + QH],
                        au[:],
                        bc[:],
                        op=MULT,
                    )

            # ---- phase C: leftover filler + outproj rows 1024-2047 ----
            while filler:
                unit = filler.pop(0)
                unit[2]()
            for sb in range(8, NS):
                for jc in range(2):
                    o_group(sb, jc)

    nc.compile()
    return nc


def _get_nc(dt_mode):
    if dt_mode not in _CACHE:
        _CACHE[dt_mode] = _build(dt_mode)
    return _CACHE[dt_mode]


def make_in_maps(x, Wq_w, Wq_b, Wk_w, Wk_b, Wv_w, Wv_b, Wo_w, Wo_b, np_dt):
    in_maps = []
    for core in range(N_CORES):
        b, half = core // 2, core % 2
        sl = slice(half * DH, (half + 1) * DH)
        in_maps.append({
            "xT": np.ascontiguousarray(x[b].T).astype(np_dt),
            "wq": np.ascontiguousarray(Wq_w[:, sl]).astype(np_dt),
            "wk": np.ascontiguousarray(Wk_w[:, sl]).astype(np_dt),
            "wv": np.ascontiguousarray(Wv_w[:, sl]).astype(np_dt),
            "wo": np.ascontiguousarray(Wo_w[sl, :]).astype(np_dt),
            "bq": np.ascontiguousarray(Wq_b[sl].reshape(-1, 128).T),
            "bk": np.ascontiguousarray(Wk_b[sl].reshape(-1, 128).T),
            "bv": np.broadcast_to(Wv_b[sl], (128, DH)).copy(),
        })
    return in_maps


def kernel(x, Wq_w, Wq_b, Wk_w, Wk_b, Wv_w, Wv_b, Wo_w, Wo_b):
    from concourse.bass_utils import run_bass_kernel_spmd

    np_dt = ml_dtypes.bfloat16 if DT_MODE == "bf16" else np.float32

    args = [np.asarray(a, np.float32) for a in
            (x, Wq_w, Wq_b, Wk_w, Wk_b, Wv_w, Wv_b, Wo_w, Wo_b)]
    x, Wq_w, Wq_b, Wk_w, Wk_b, Wv_w, Wv_b, Wo_w, Wo_b = args

    nc = _get_nc(DT_MODE)
    in_maps = make_in_maps(x, Wq_w, Wq_b, Wk_w, Wk_b, Wv_w, Wv_b, Wo_w, Wo_b,
                           np_dt)
    res = run_bass_kernel_spmd(nc, in_maps, list(range(N_CORES)))

    out = np.empty((B, S, D), np.float32)
    for b in range(B):
        out[b] = (res.results[2 * b]["out"].astype(np.float32)
                  + res.results[2 * b + 1]["out"].astype(np.float32) + Wo_b)
    return out


# revision 15
# speedup vs baseline: 1.0122x; 1.0122x over previous
"""Trainium2 Bass kernel for nn_MultiHeadAttention (B=4, S=2048, D=1024, H=16).

Sharding: 8 cores = batch(4) x head-half(2).  Each core computes, for its
batch element, 8 of the 16 heads: QKV projections against column-sliced
weights, causal attention, and the output projection against the matching
row-slice of Wo.  The two bf16 partial outputs per batch element are summed
on the host (replaces the tensor-parallel all-reduce), and Wo_b is added
there.

v4 design notes:
- Attention scores use the transposed layout scoresT[k, q]; the softmax
  denominator comes from an all-ones column prepended to each head's V
  block (so it lands on psum partition 0, where the fast reciprocal can
  read it straight out of PSUM); normalization runs off the critical path.
- The K=64 scores matmuls are row-packed in the PE array: even kb targets
  array rows 0-63, odd kb rows 64-127 (explicit tile_position), using
  half-swapped copies qT2/kT2 so the operands sit on the matching SBUF
  partitions.  Adjacent even/odd scores matmuls run concurrently -> ~2x
  scores throughput.
- exp on the Scalar engine is the attention-phase serial bottleneck
  (~166us).  The PE stream is kept stall-free (idle gaps re-throttle the
  PE clock to 1.2 GHz): scores for kb-pair g+1 are emitted one group ahead
  of the exps of pair g, and projection / output-projection matmul groups
  are paced into the attention stream by a cycle ledger so the PE always
  has dependency-free work while ACT churns through the exps.
"""

import sys

if "/opt/trn_rl_repo" not in sys.path:
    sys.path.insert(0, "/opt/trn_rl_repo")

import numpy as np
import ml_dtypes

B, S, D = 4, 2048, 1024
H, HD = 16, 64
HH = H // 2          # heads per core
DH = D // 2          # local attention feature dim (HH * HD)
N_CORES = 8
QH = 1024            # q-range processed per attention pass (psum budget)

DT_MODE = "bf16"

_CACHE = {}


def _build(dt_mode):
    import concourse.bass as bass
    import concourse.mybir as mybir
    from concourse import bacc
    from concourse.tile import TileContext
    from concourse.masks import make_upper_triangular

    F32 = mybir.dt.float32
    DT = mybir.dt.bfloat16 if dt_mode == "bf16" else mybir.dt.float32

    ADD = mybir.AluOpType.add
    MULT = mybir.AluOpType.mult
    EXP = mybir.ActivationFunctionType.Exp

    nc = bacc.Bacc("TRN2", target_bir_lowering=False, debug=False,
                   num_devices=N_CORES)

    xT = nc.dram_tensor("xT", [D, S], DT, kind="ExternalInput").ap()
    wq = nc.dram_tensor("wq", [D, DH], DT, kind="ExternalInput").ap()
    wk = nc.dram_tensor("wk", [D, DH], DT, kind="ExternalInput").ap()
    wv = nc.dram_tensor("wv", [D, DH], DT, kind="ExternalInput").ap()
    wo = nc.dram_tensor("wo", [DH, D], DT, kind="ExternalInput").ap()
    bq = nc.dram_tensor("bq", [128, DH // 128], F32, kind="ExternalInput").ap()
    bk = nc.dram_tensor("bk", [128, DH // 128], F32, kind="ExternalInput").ap()
    bv = nc.dram_tensor("bv", [128, DH], F32, kind="ExternalInput").ap()
    out = nc.dram_tensor("out", [S, D], DT, kind="ExternalOutput").ap()

    ND = D // 128        # 8 contraction tiles over D
    NS = S // 128        # 16 s-blocks
    NJ = DH // 128       # 4 feature tiles of the local 512 dim
    NSC = S // 512       # 4 columns of 512 over S

    from contextlib import ExitStack

    with TileContext(nc) as tc:
        with ExitStack() as stack:
            pool = lambda *a, **kw: stack.enter_context(tc.tile_pool(*a, **kw))
            pp = pool(name="persist", bufs=1)
            pqT = pool(name="qT", bufs=NJ)
            pkT = pool(name="kT", bufs=NJ)
            pqT2 = pool(name="qT2", bufs=NJ)
            pkT2 = pool(name="kT2", bufs=NJ)
            pv = pool(name="vaug", bufs=NS)
            pattnT = pool(name="attnT", bufs=NJ)
            pxt = pool(name="xt", bufs=ND)
            pw = pool(name="w", bufs=3 * ND)
            pwo = pool(name="wo", bufs=NJ)
            pproj = pool(name="projps", bufs=2, space="PSUM")
            pexp = pool(name="exp", bufs=4)
            pau = pool(name="au", bufs=2)
            pof = pool(name="of", bufs=2)
            prc = pool(name="recip", bufs=2)
            pscps = pool(name="scps", bufs=2, space="PSUM")
            patps = pool(name="atps", bufs=1, space="PSUM")
            # ---- constants / biases ----
            bq_t = pp.tile([128, NJ], F32, tag="bq")
            nc.sync.dma_start(bq_t[:], bq[:])
            bk_t = pp.tile([128, NJ], F32, tag="bk")
            nc.sync.dma_start(bk_t[:], bk[:])
            bv_t = pp.tile([128, DH], F32, tag="bv")
            nc.sync.dma_start(bv_t[:], bv[:])
            ones_t = pp.tile([128, HH], F32, tag="ones")
            nc.gpsimd.memset(ones_t[:], 1.0)
            # causal mask for diagonal 128x128 squares of scoresT[k, q]:
            # valid (k <= q) <=> partition p <= free f -> upper-tri incl diag.
            mask_f = pp.tile([128, 128], F32, tag="maskf")
            make_upper_triangular(nc, mask_f[:], val=1.0, diag=True)
            mask_t = pp.tile([128, 128], DT, tag="mask")
            nc.vector.tensor_copy(mask_t[:], mask_f[:])

            # persistent activation buffers
            qT_t = [pqT.tile([128, S], DT, tag="qT", name=f"qT{i}")
                    for i in range(NJ)]
            kT_t = [pkT.tile([128, S], DT, tag="kT", name=f"kT{i}")
                    for i in range(NJ)]
            qT2_t = [pqT2.tile([128, S], DT, tag="qT2", name=f"qT2{i}")
                     for i in range(NJ)]
            kT2_t = [pkT2.tile([128, S], DT, tag="kT2", name=f"kT2{i}")
                     for i in range(NJ)]
            v_t = [pv.tile([128, HH * (HD + 1)], DT, tag="vaug",
                           name=f"vaug{i}") for i in range(NS)]
            aT_t = [pattnT.tile([128, S], DT, tag="attnT", name=f"attnT{i}")
                    for i in range(NJ)]

            # input DMAs, ordered so the first V matmuls start early
            xt_t = [pxt.tile([128, S], DT, tag="xt", name=f"xt{i}")
                    for i in range(ND)]
            wv_t = [pw.tile([128, DH], DT, tag="w3", name=f"wv{db}")
                    for db in range(ND)]
            wq_t = [pw.tile([128, DH], DT, tag="w3", name=f"wq{db}")
                    for db in range(ND)]
            wk_t = [pw.tile([128, DH], DT, tag="w3", name=f"wk{db}")
                    for db in range(ND)]
            wo_t = [pwo.tile([128, D], DT, tag="wo", name=f"wo{db}")
                    for db in range(NJ)]

            def dma_w(w_t, ap):
                for db in range(ND):
                    nc.sync.dma_start(w_t[db][:], ap[db * 128:(db + 1) * 128, :])

            def dma_x(cc):
                cs = slice(cc * 512, (cc + 1) * 512)
                for db in range(ND):
                    nc.sync.dma_start(xt_t[db][:, cs],
                                      xT[db * 128:(db + 1) * 128, cs])

            dma_w(wv_t, wv)
            dma_x(0)
            dma_x(1)
            dma_w(wq_t, wq)
            dma_x(2)
            dma_x(3)
            dma_w(wk_t, wk)
            for db in range(NJ):
                nc.sync.dma_start(wo_t[db][:], wo[db * 128:(db + 1) * 128, :])

            # ---- HAM warm-up: ~60 dummy matmuls on the first-arriving
            # weight tiles keep the PE busy through the DMA lead-in so the
            # clock gate is already at 8/8 when real work starts.
            warm_ps = pproj.tile([128, 512], F32, tag="proj", name="warm")
            for i in range(60):
                db = i % ND
                nc.tensor.matmul(warm_ps[:], lhsT=wv_t[db][:, 0:128],
                                 rhs=wv_t[db][:], start=True, stop=True)

            # ---- work-unit emitters (each ~850ns of PE work) ----
            live_ps = {}

            def v_half(sb, half):
                name = f"psv{sb}"
                if half == 0:
                    ps = pproj.tile([128, 512], F32, tag="proj", name=name)
                    live_ps[name] = ps
                else:
                    ps = live_ps.pop(name)
                for db in range(4 * half, 4 * half + 4):
                    nc.tensor.matmul(
                        ps[:],
                        lhsT=xt_t[db][:, sb * 128:(sb + 1) * 128],
                        rhs=wv_t[db][:],
                        start=(db == 0), stop=(db == ND - 1),
                    )
                if half == 1:
                    # v_aug layout per head: [v+bias | ones] (65 cols); the
                    # ones column makes psum row 64 the softmax denominator.
                    vt = v_t[sb]
                    v3 = vt[:].rearrange("p (h e) -> p h e", e=HD + 1)
                    nc.vector.tensor_tensor(
                        v3[:, :, 0:HD],
                        ps[:].rearrange("p (h e) -> p h e", e=HD),
                        bv_t[:].rearrange("p (h e) -> p h e", e=HD),
                        op=ADD,
                    )
                    nc.vector.tensor_copy(
                        v3[:, :, HD:HD + 1],
                        ones_t[:].rearrange("p (h e) -> p h e", e=1),
                    )

            def qk_half(w_t, bias_t, dstT, jb, sc, half):
                name = f"ps{'q' if dstT is qT_t else 'k'}{jb}_{sc}"
                if half == 0:
                    ps = pproj.tile([128, 512], F32, tag="proj", name=name)
                    live_ps[name] = ps
                else:
                    ps = live_ps.pop(name)
                for db in range(4 * half, 4 * half + 4):
                    nc.tensor.matmul(
                        ps[:],
                        lhsT=w_t[db][:, jb * 128:(jb + 1) * 128],
                        rhs=xt_t[db][:, sc * 512:(sc + 1) * 512],
                        start=(db == 0), stop=(db == ND - 1),
                    )
                if half == 1:
                    nc.vector.tensor_scalar_add(
                        dstT[jb][:, sc * 512:(sc + 1) * 512],
                        ps[:], bias_t[:, jb:jb + 1],
                    )

            def swap_copy(jb, c0=0, c1=S):
                # half-swapped copies so odd-kb scores matmuls can target
                # the other PE row group
                for src, dst in ((qT_t[jb], qT2_t[jb]), (kT_t[jb], kT2_t[jb])):
                    nc.vector.tensor_copy(dst[0:64, c0:c1], src[64:128, c0:c1])
                    nc.vector.tensor_copy(dst[64:128, c0:c1], src[0:64, c0:c1])

            def o_group(sb, jc):
                ps = pproj.tile([128, 512], F32, tag="proj",
                                name=f"pso{sb}_{jc}")
                for db in range(NJ):
                    nc.tensor.matmul(
                        ps[:],
                        lhsT=aT_t[db][:, sb * 128:(sb + 1) * 128],
                        rhs=wo_t[db][:, jc * 512:(jc + 1) * 512],
                        start=(db == 0), stop=(db == NJ - 1),
                    )
                ot = pof.tile([128, 512], DT, tag="of", name=f"ot{sb}_{jc}")
                nc.vector.tensor_copy(ot[:], ps[:])
                nc.sync.dma_start(
                    out[sb * 128:(sb + 1) * 128, jc * 512:(jc + 1) * 512],
                    ot[:],
                )

            # filler queue: (req_key, min_key, pe_ns, closure).
            # req_key = (gidx, g) by which the unit MUST have been emitted
            # (dependency order); gidx = 2*h + qh, g = kb-pair index within
            # that (h, qh).  min_key: earliest (gidx, g) at which the unit
            # MAY run (outproj needs all heads' qh=0 rows written AND the
            # normalize chain drained; staggering avoids head-of-line
            # blocking the PE on a late aT write).
            ZERO = (0, 0)
            filler = []
            for sb in range(4, 8):           # v for kb 4-7: jit in h0/qh0
                for half in range(2):
                    filler.append(((0, sb // 2), ZERO, 853,
                                   lambda s=sb, hf=half: v_half(s, hf)))
            for sc in (2, 3):                # qT jb0 cols 1024-2047: h0/qh1
                for half in range(2):
                    filler.append(((1, 0), ZERO, 853,
                                   lambda s=sc, hf=half:
                                      qk_half(wq_t, bq_t, qT_t, 0, s, hf)))
            filler.append(((1, 0), ZERO, 0, lambda: swap_copy(0, 1024, 2048)))
            for sc in (2, 3):                # kT jb0 rows 1024-2047: h0/qh1
                for half in range(2):
                    filler.append(((1, 4), ZERO, 853,
                                   lambda s=sc, hf=half:
                                      qk_half(wk_t, bk_t, kT_t, 0, s, hf)))
            for sb in range(8, NS):          # v for kb 8-15: jit in h0/qh1
                for half in range(2):
                    filler.append(((1, sb // 2), ZERO, 853,
                                   lambda s=sb, hf=half: v_half(s, hf)))
            for jb in range(1, NJ):          # qk for jb: by h=2*jb
                for sc in range(NSC):
                    for w_t, bias_t, dstT in ((wq_t, bq_t, qT_t),
                                              (wk_t, bk_t, kT_t)):
                        for half in range(2):
                            filler.append(
                                ((4 * jb, 0), ZERO, 853,
                                 lambda w=w_t, b=bias_t, d=dstT, j=jb,
                                        s=sc, hf=half:
                                    qk_half(w, b, d, j, s, hf)))
                filler.append(((4 * jb, 0), ZERO, 0,
                               lambda j=jb: swap_copy(j)))
            i = 0                            # outproj rows 0-1023: late h7/qh1
            for sb in range(8):
                for jc in range(2):
                    filler.append(((99, 0), (15, 3 + i // 4), 853,
                                   lambda s=sb, j=jc: o_group(s, j)))
                    i += 1

            debt = [0.0]

            def emit_filler_by_debt(cur_key):
                while debt[0] > 900 and filler and filler[0][1] <= cur_key:
                    unit = filler.pop(0)
                    unit[3]()
                    debt[0] -= unit[2]
                debt[0] = min(debt[0], 5000.0)

            def force_filler(key):
                while filler and filler[0][0] <= key:
                    unit = filler.pop(0)
                    unit[3]()
                    debt[0] -= unit[2]

            # ---- phase A: V(sb 0-3) + QK(jb=0, cols 0-1023) ----
            for sb in range(4):
                v_half(sb, 0)
                v_half(sb, 1)
            for sc in (0, 1):
                for w_t, bias_t, dstT in ((wq_t, bq_t, qT_t),
                                          (wk_t, bk_t, kT_t)):
                    qk_half(w_t, bias_t, dstT, 0, sc, 0)
                    qk_half(w_t, bias_t, dstT, 0, sc, 1)
            swap_copy(0, 0, 1024)

            # ---- phase B: attention ----
            def chunk_cols(lo):
                chunks = []
                c = lo
                while c < QH:
                    c1 = min((c // 512 + 1) * 512, QH)
                    chunks.append((c, c1))
                    c = c1
                return chunks

            for h in range(HH):
                hb, hr = h // 2, (h % 2) * 64
                hr2 = 64 - hr
                vcol = h * (HD + 1)
                for qh in range(S // QH):
                    gidx = 2 * h + qh
                    q0 = qh * QH
                    at = patps.tile([65, QH], F32, tag="at",
                                    name=f"at{h}_{qh}")
                    nkb = (q0 + QH) // 128

                    def scores(kb):
                        k0 = kb * 128
                        lo = max(k0 - q0, 0)
                        sc = pscps.tile([128, QH], F32, tag="sc",
                                        name=f"sc{h}_{qh}_{kb}")
                        if kb % 2 == 0:
                            kT, qT, rp = kT_t[hb], qT_t[hb], hr
                        else:
                            kT, qT, rp = kT2_t[hb], qT2_t[hb], hr2
                        for (c0, c1) in chunk_cols(lo):
                            nc.tensor.matmul(
                                sc[:, c0:c1],
                                lhsT=kT[rp:rp + 64, k0:k0 + 128],
                                rhs=qT[rp:rp + 64, q0 + c0:q0 + c1],
                                start=True, stop=True,
                                tile_position=(rp, 0),
                            )
                        return sc

                    def exp(kb, sc):
                        k0 = kb * 128
                        lo = max(k0 - q0, 0)
                        et = pexp.tile([128, QH], DT, tag="exp",
                                       name=f"et{h}_{qh}_{kb}")
                        nc.scalar.activation(et[:, lo:QH], sc[:, lo:QH],
                                             EXP, scale=1.0 / np.sqrt(HD))
                        if k0 >= q0:
                            nc.vector.tensor_mul(et[:, lo:lo + 128],
                                                 et[:, lo:lo + 128],
                                                 mask_t[:])
                        return et

                    def pv(kb, et):
                        lo = max(kb * 128 - q0, 0)
                        for (c0, c1) in chunk_cols(lo):
                            nc.tensor.matmul(
                                at[0:65, c0:c1],
                                lhsT=v_t[kb][:, vcol:vcol + HD + 1],
                                rhs=et[:, c0:c1],
                                start=(kb == 0),
                                stop=(kb == (q0 + c1 - 1) // 128),
                            )

                    def est(kb):
                        lo = max(kb * 128 - q0, 0)
                        n = QH - lo
                        return (n + 352) / 1.2, n / 2.4

                    # Emission order per pair-group g.  The PE queue is
                    # in-order, so anything emitted ahead of ready work
                    # head-of-line-blocks it; the PVs therefore LAG one
                    # group (their exps finished last group), filler comes
                    # next (dependency-free), and the lookahead scores pair
                    # goes last (its sc-slot WAR on this group's exps has
                    # cleared by the time the PE drains the earlier work).
                    #   ACT: exp(2g), exp(2g+1)
                    #   PE : pv(2g-2), pv(2g-1) | filler | scores(2g+2/3)
                    npair = nkb // 2
                    force_filler((gidx, 0))
                    sc_pair = [scores(0), scores(1)]
                    prev_ets = None
                    for g in range(npair):
                        et0 = exp(2 * g, sc_pair[0])
                        et1 = exp(2 * g + 1, sc_pair[1])
                        if prev_ets is not None:
                            pv(2 * g - 2, prev_ets[0])
                            pv(2 * g - 1, prev_ets[1])
                        a0, p0 = est(2 * g)
                        a1, p1 = est(2 * g + 1)
                        # scores pair is concurrent (~p0), PVs are serial
                        debt[0] += (a0 + a1) - (p0 + p0 + p1)
                        emit_filler_by_debt((gidx, g))
                        if g + 1 < npair:
                            force_filler((gidx, g + 1))
                            sc_pair = [scores(2 * g + 2), scores(2 * g + 3)]
                        prev_ets = (et0, et1)
                    pv(nkb - 2, prev_ets[0])
                    pv(nkb - 1, prev_ets[1])

                    # drain + normalize off the critical path (PSUM/SBUF
                    # partition bases must be 32-aligned, so two copies).
                    au = pau.tile([64, QH], F32, tag="au", name=f"au{h}_{qh}")
                    nc.vector.tensor_copy(au[:], at[0:64, :])
                    dn = prc.tile([1, QH], F32, tag="dn", name=f"dn{h}_{qh}")
                    nc.vector.tensor_copy(dn[:], at[64:65, :])
                    rc = prc.tile([1, QH], F32, tag="rc", name=f"rc{h}_{qh}")
                    nc.vector.reciprocal_approx_fast(rc[:], dn[:])
                    bc = prc.tile([64, QH], F32, tag="bc", name=f"bc{h}_{qh}")
                    nc.gpsimd.partition_broadcast(bc[:], rc[:])
                    nc.gpsimd.tensor_tensor(
                        aT_t[hb][hr:hr + 64, q0:q0 + QH],
                        au[:],
                        bc[:],
                        op=MULT,
                    )

            # ---- phase C: leftover filler + outproj rows 1024-2047 ----
            while filler:
                unit = filler.pop(0)
                unit[3]()
            for sb in range(8, NS):
                for jc in range(2):
                    o_group(sb, jc)

    nc.compile()
    return nc


def _get_nc(dt_mode):
    if dt_mode not in _CACHE:
        _CACHE[dt_mode] = _build(dt_mode)
    return _CACHE[dt_mode]


def make_in_maps(x, Wq_w, Wq_b, Wk_w, Wk_b, Wv_w, Wv_b, Wo_w, Wo_b, np_dt):
    in_maps = []
    for core in range(N_CORES):
        b, half = core // 2, core % 2
        sl = slice(half * DH, (half + 1) * DH)
        in_maps.append({
            "xT": np.ascontiguousarray(x[b].T).astype(np_dt),
            "wq": np.ascontiguousarray(Wq_w[:, sl]).astype(np_dt),
            "wk": np.ascontiguousarray(Wk_w[:, sl]).astype(np_dt),
            "wv": np.ascontiguousarray(Wv_w[:, sl]).astype(np_dt),
            "wo": np.ascontiguousarray(Wo_w[sl, :]).astype(np_dt),
            "bq": np.ascontiguousarray(Wq_b[sl].reshape(-1, 128).T),
            "bk": np.ascontiguousarray(Wk_b[sl].reshape(-1, 128).T),
            "bv": np.broadcast_to(Wv_b[sl], (128, DH)).copy(),
        })
    return in_maps


def kernel(x, Wq_w, Wq_b, Wk_w, Wk_b, Wv_w, Wv_b, Wo_w, Wo_b):
    from concourse.bass_utils import run_bass_kernel_spmd

    np_dt = ml_dtypes.bfloat16 if DT_MODE == "bf16" else np.float32

    args = [np.asarray(a, np.float32) for a in
            (x, Wq_w, Wq_b, Wk_w, Wk_b, Wv_w, Wv_b, Wo_w, Wo_b)]
    x, Wq_w, Wq_b, Wk_w, Wk_b, Wv_w, Wv_b, Wo_w, Wo_b = args

    nc = _get_nc(DT_MODE)
    in_maps = make_in_maps(x, Wq_w, Wq_b, Wk_w, Wk_b, Wv_w, Wv_b, Wo_w, Wo_b,
                           np_dt)
    res = run_bass_kernel_spmd(nc, in_maps, list(range(N_CORES)))

    out = np.empty((B, S, D), np.float32)
    for b in range(B):
        out[b] = (res.results[2 * b]["out"].astype(np.float32)
                  + res.results[2 * b + 1]["out"].astype(np.float32) + Wo_b)
    return out


# revision 18
# speedup vs baseline: 1.0161x; 1.0038x over previous
"""Trainium2 Bass kernel for nn_MultiHeadAttention (B=4, S=2048, D=1024, H=16).

Sharding: 8 cores = batch(4) x head-half(2).  Each core computes, for its
batch element, 8 of the 16 heads: QKV projections against column-sliced
weights, causal attention, and the output projection against the matching
row-slice of Wo.  The two bf16 partial outputs per batch element are summed
on the host (replaces the tensor-parallel all-reduce), and Wo_b is added
there.

v4 design notes:
- Attention scores use the transposed layout scoresT[k, q]; the softmax
  denominator comes from an all-ones column prepended to each head's V
  block (so it lands on psum partition 0, where the fast reciprocal can
  read it straight out of PSUM); normalization runs off the critical path.
- The K=64 scores matmuls are row-packed in the PE array: even kb targets
  array rows 0-63, odd kb rows 64-127 (explicit tile_position), using
  half-swapped copies qT2/kT2 so the operands sit on the matching SBUF
  partitions.  Adjacent even/odd scores matmuls run concurrently -> ~2x
  scores throughput.
- exp on the Scalar engine is the attention-phase serial bottleneck
  (~166us).  The PE stream is kept stall-free (idle gaps re-throttle the
  PE clock to 1.2 GHz): scores for kb-pair g+1 are emitted one group ahead
  of the exps of pair g, and projection / output-projection matmul groups
  are paced into the attention stream by a cycle ledger so the PE always
  has dependency-free work while ACT churns through the exps.
"""

import sys

if "/opt/trn_rl_repo" not in sys.path:
    sys.path.insert(0, "/opt/trn_rl_repo")

import numpy as np
import ml_dtypes

B, S, D = 4, 2048, 1024
H, HD = 16, 64
HH = H // 2          # heads per core
DH = D // 2          # local attention feature dim (HH * HD)
N_CORES = 8
QH = 1024            # q-range processed per attention pass (psum budget)

DT_MODE = "bf16"

_CACHE = {}


def _build(dt_mode):
    import concourse.bass as bass
    import concourse.mybir as mybir
    from concourse import bacc
    from concourse.tile import TileContext
    from concourse.masks import make_upper_triangular

    F32 = mybir.dt.float32
    DT = mybir.dt.bfloat16 if dt_mode == "bf16" else mybir.dt.float32

    ADD = mybir.AluOpType.add
    MULT = mybir.AluOpType.mult
    EXP = mybir.ActivationFunctionType.Exp

    nc = bacc.Bacc("TRN2", target_bir_lowering=False, debug=False,
                   num_devices=N_CORES)

    xT = nc.dram_tensor("xT", [D, S], DT, kind="ExternalInput").ap()
    wq = nc.dram_tensor("wq", [D, DH], DT, kind="ExternalInput").ap()
    wk = nc.dram_tensor("wk", [D, DH], DT, kind="ExternalInput").ap()
    wv = nc.dram_tensor("wv", [D, DH], DT, kind="ExternalInput").ap()
    wo = nc.dram_tensor("wo", [DH, D], DT, kind="ExternalInput").ap()
    bq = nc.dram_tensor("bq", [128, DH // 128], F32, kind="ExternalInput").ap()
    bk = nc.dram_tensor("bk", [128, DH // 128], F32, kind="ExternalInput").ap()
    bv = nc.dram_tensor("bv", [128, DH], F32, kind="ExternalInput").ap()
    out = nc.dram_tensor("out", [S, D], DT, kind="ExternalOutput").ap()

    ND = D // 128        # 8 contraction tiles over D
    NS = S // 128        # 16 s-blocks
    NJ = DH // 128       # 4 feature tiles of the local 512 dim
    NSC = S // 512       # 4 columns of 512 over S

    from contextlib import ExitStack

    with TileContext(nc) as tc:
        with ExitStack() as stack:
            pool = lambda *a, **kw: stack.enter_context(tc.tile_pool(*a, **kw))
            pp = pool(name="persist", bufs=1)
            pqT = pool(name="qT", bufs=NJ)
            pkT = pool(name="kT", bufs=NJ)
            pqT2 = pool(name="qT2", bufs=NJ)
            pkT2 = pool(name="kT2", bufs=NJ)
            pv = pool(name="vaug", bufs=NS)
            pattnT = pool(name="attnT", bufs=NJ)
            pxt = pool(name="xt", bufs=ND)
            pw = pool(name="w", bufs=3 * ND)
            pwo = pool(name="wo", bufs=NJ)
            pproj = pool(name="projps", bufs=2, space="PSUM")
            pexp = pool(name="exp", bufs=4)
            pau = pool(name="au", bufs=2)
            pof = pool(name="of", bufs=2)
            prc = pool(name="recip", bufs=2)
            pscps = pool(name="scps", bufs=2, space="PSUM")
            patps = pool(name="atps", bufs=1, space="PSUM")
            # ---- constants / biases ----
            bq_t = pp.tile([128, NJ], F32, tag="bq")
            nc.sync.dma_start(bq_t[:], bq[:])
            bk_t = pp.tile([128, NJ], F32, tag="bk")
            nc.sync.dma_start(bk_t[:], bk[:])
            bv_t = pp.tile([128, DH], F32, tag="bv")
            nc.sync.dma_start(bv_t[:], bv[:])
            ones_t = pp.tile([128, HH], F32, tag="ones")
            nc.gpsimd.memset(ones_t[:], 1.0)
            # causal mask for diagonal 128x128 squares of scoresT[k, q]:
            # valid (k <= q) <=> partition p <= free f -> upper-tri incl diag.
            mask_f = pp.tile([128, 128], F32, tag="maskf")
            make_upper_triangular(nc, mask_f[:], val=1.0, diag=True)
            mask_t = pp.tile([128, 128], DT, tag="mask")
            nc.vector.tensor_copy(mask_t[:], mask_f[:])

            # persistent activation buffers
            qT_t = [pqT.tile([128, S], DT, tag="qT", name=f"qT{i}")
                    for i in range(NJ)]
            kT_t = [pkT.tile([128, S], DT, tag="kT", name=f"kT{i}")
                    for i in range(NJ)]
            qT2_t = [pqT2.tile([128, S], DT, tag="qT2", name=f"qT2{i}")
                     for i in range(NJ)]
            kT2_t = [pkT2.tile([128, S], DT, tag="kT2", name=f"kT2{i}")
                     for i in range(NJ)]
            v_t = [pv.tile([128, HH * (HD + 1)], DT, tag="vaug",
                           name=f"vaug{i}") for i in range(NS)]
            aT_t = [pattnT.tile([128, S], DT, tag="attnT", name=f"attnT{i}")
                    for i in range(NJ)]

            # input DMAs, ordered so the first V matmuls start early
            xt_t = [pxt.tile([128, S], DT, tag="xt", name=f"xt{i}")
                    for i in range(ND)]
            wv_t = [pw.tile([128, DH], DT, tag="w3", name=f"wv{db}")
                    for db in range(ND)]
            wq_t = [pw.tile([128, DH], DT, tag="w3", name=f"wq{db}")
                    for db in range(ND)]
            wk_t = [pw.tile([128, DH], DT, tag="w3", name=f"wk{db}")
                    for db in range(ND)]
            wo_t = [pwo.tile([128, D], DT, tag="wo", name=f"wo{db}")
                    for db in range(NJ)]

            def dma_w(w_t, ap):
                for db in range(ND):
                    nc.sync.dma_start(w_t[db][:], ap[db * 128:(db + 1) * 128, :])

            def dma_x(cc):
                cs = slice(cc * 512, (cc + 1) * 512)
                for db in range(ND):
                    nc.sync.dma_start(xt_t[db][:, cs],
                                      xT[db * 128:(db + 1) * 128, cs])

            dma_w(wv_t, wv)
            dma_x(0)
            dma_x(1)
            dma_w(wq_t, wq)
            dma_x(2)
            dma_x(3)
            dma_w(wk_t, wk)
            for db in range(NJ):
                nc.sync.dma_start(wo_t[db][:], wo[db * 128:(db + 1) * 128, :])

            # ---- HAM warm-up: ~60 dummy matmuls on the first-arriving
            # weight tiles keep the PE busy through the DMA lead-in so the
            # clock gate is already at 8/8 when real work starts.
            warm_ps = pproj.tile([128, 512], F32, tag="proj", name="warm")
            for i in range(60):
                db = i % ND
                nc.tensor.matmul(warm_ps[:], lhsT=wv_t[db][:, 0:128],
                                 rhs=wv_t[db][:], start=True, stop=True)

            # ---- work-unit emitters (each ~850ns of PE work) ----
            live_ps = {}

            def v_half(sb, half):
                name = f"psv{sb}"
                if half == 0:
                    ps = pproj.tile([128, 512], F32, tag="proj", name=name)
                    live_ps[name] = ps
                else:
                    ps = live_ps.pop(name)
                for db in range(4 * half, 4 * half + 4):
                    nc.tensor.matmul(
                        ps[:],
                        lhsT=xt_t[db][:, sb * 128:(sb + 1) * 128],
                        rhs=wv_t[db][:],
                        start=(db == 0), stop=(db == ND - 1),
                    )
                if half == 1:
                    # v_aug layout per head: [v+bias | ones] (65 cols); the
                    # ones column makes psum row 64 the softmax denominator.
                    vt = v_t[sb]
                    v3 = vt[:].rearrange("p (h e) -> p h e", e=HD + 1)
                    nc.vector.tensor_tensor(
                        v3[:, :, 0:HD],
                        ps[:].rearrange("p (h e) -> p h e", e=HD),
                        bv_t[:].rearrange("p (h e) -> p h e", e=HD),
                        op=ADD,
                    )
                    nc.vector.tensor_copy(
                        v3[:, :, HD:HD + 1],
                        ones_t[:].rearrange("p (h e) -> p h e", e=1),
                    )

            def qk_half(w_t, bias_t, dstT, jb, sc, half):
                name = f"ps{'q' if dstT is qT_t else 'k'}{jb}_{sc}"
                if half == 0:
                    ps = pproj.tile([128, 512], F32, tag="proj", name=name)
                    live_ps[name] = ps
                else:
                    ps = live_ps.pop(name)
                for db in range(4 * half, 4 * half + 4):
                    nc.tensor.matmul(
                        ps[:],
                        lhsT=w_t[db][:, jb * 128:(jb + 1) * 128],
                        rhs=xt_t[db][:, sc * 512:(sc + 1) * 512],
                        start=(db == 0), stop=(db == ND - 1),
                    )
                if half == 1:
                    nc.vector.tensor_scalar_add(
                        dstT[jb][:, sc * 512:(sc + 1) * 512],
                        ps[:], bias_t[:, jb:jb + 1],
                    )

            def swap_one(src, dst, c0, c1):
                nc.vector.tensor_copy(dst[0:64, c0:c1], src[64:128, c0:c1])
                nc.vector.tensor_copy(dst[64:128, c0:c1], src[0:64, c0:c1])

            def swap_copy(jb, c0=0, c1=S):
                # half-swapped copies so odd-kb scores matmuls can target
                # the other PE row group
                swap_one(qT_t[jb], qT2_t[jb], c0, c1)
                swap_one(kT_t[jb], kT2_t[jb], c0, c1)

            def o_group(sb, jc):
                ps = pproj.tile([128, 512], F32, tag="proj",
                                name=f"pso{sb}_{jc}")
                for db in range(NJ):
                    nc.tensor.matmul(
                        ps[:],
                        lhsT=aT_t[db][:, sb * 128:(sb + 1) * 128],
                        rhs=wo_t[db][:, jc * 512:(jc + 1) * 512],
                        start=(db == 0), stop=(db == NJ - 1),
                    )
                ot = pof.tile([128, 512], DT, tag="of", name=f"ot{sb}_{jc}")
                nc.vector.tensor_copy(ot[:], ps[:])
                nc.sync.dma_start(
                    out[sb * 128:(sb + 1) * 128, jc * 512:(jc + 1) * 512],
                    ot[:],
                )

            # filler queue: (req_key, min_key, pe_ns, closure).
            # req_key = (gidx, g) by which the unit MUST have been emitted
            # (dependency order); gidx = 2*h + qh, g = kb-pair index within
            # that (h, qh).  min_key: earliest (gidx, g) at which the unit
            # MAY run (outproj needs all heads' qh=0 rows written AND the
            # normalize chain drained; staggering avoids head-of-line
            # blocking the PE on a late aT write).
            ZERO = (0, 0)
            filler = []
            for sb in range(4, 8):           # v for kb 4-7: jit in h0/qh0
                for half in range(2):
                    filler.append(((0, sb // 2), ZERO, 853,
                                   lambda s=sb, hf=half: v_half(s, hf)))
            for sc in (2, 3):                # qT jb0 cols 1024-2047: h0/qh1
                for half in range(2):
                    filler.append(((1, 0), ZERO, 853,
                                   lambda s=sc, hf=half:
                                      qk_half(wq_t, bq_t, qT_t, 0, s, hf)))
            filler.append(((1, 0), ZERO, 0,
                           lambda: swap_one(qT_t[0], qT2_t[0], 1024, 2048)))
            for sc in (2, 3):                # kT jb0 rows 1024-2047: h0/qh1
                for half in range(2):
                    filler.append(((1, 4), ZERO, 853,
                                   lambda s=sc, hf=half:
                                      qk_half(wk_t, bk_t, kT_t, 0, s, hf)))
            filler.append(((1, 4), ZERO, 0,
                           lambda: swap_one(kT_t[0], kT2_t[0], 1024, 2048)))
            for sb in range(8, NS):          # v for kb 8-15: jit in h0/qh1
                for half in range(2):
                    filler.append(((1, sb // 2), ZERO, 853,
                                   lambda s=sb, hf=half: v_half(s, hf)))
            for jb in range(1, NJ):          # qk for jb: by h=2*jb
                for sc in range(NSC):
                    for w_t, bias_t, dstT in ((wq_t, bq_t, qT_t),
                                              (wk_t, bk_t, kT_t)):
                        for half in range(2):
                            filler.append(
                                ((4 * jb, 0), ZERO, 853,
                                 lambda w=w_t, b=bias_t, d=dstT, j=jb,
                                        s=sc, hf=half:
                                    qk_half(w, b, d, j, s, hf)))
                filler.append(((4 * jb, 0), ZERO, 0,
                               lambda j=jb: swap_copy(j)))
            i = 0                            # outproj rows 0-1023: late h7/qh1
            for sb in range(8):
                for jc in range(2):
                    filler.append(((99, 0), (15, 3 + i // 4), 853,
                                   lambda s=sb, j=jc: o_group(s, j)))
                    i += 1

            debt = [0.0]

            def emit_filler_by_debt(cur_key):
                while debt[0] > 900 and filler and filler[0][1] <= cur_key:
                    unit = filler.pop(0)
                    unit[3]()
                    debt[0] -= unit[2]
                debt[0] = min(debt[0], 5000.0)

            def force_filler(key):
                while filler and filler[0][0] <= key:
                    unit = filler.pop(0)
                    unit[3]()
                    debt[0] -= unit[2]

            # ---- phase A: V(sb 0-3) + QK(jb=0, cols 0-1023); the swaps
            # go right after their producers so they sit early in the DVE
            # queue (h0's first odd-kb scores wait on them) ----
            for sb in range(4):
                v_half(sb, 0)
                v_half(sb, 1)
            for w_t, bias_t, dstT in ((wq_t, bq_t, qT_t),
                                      (wk_t, bk_t, kT_t)):
                for sc in (0, 1):
                    qk_half(w_t, bias_t, dstT, 0, sc, 0)
                    qk_half(w_t, bias_t, dstT, 0, sc, 1)
                if dstT is qT_t:
                    swap_one(qT_t[0], qT2_t[0], 0, 1024)
                else:
                    swap_one(kT_t[0], kT2_t[0], 0, 1024)

            # ---- phase B: attention ----
            def chunk_cols(lo):
                chunks = []
                c = lo
                while c < QH:
                    c1 = min((c // 512 + 1) * 512, QH)
                    chunks.append((c, c1))
                    c = c1
                return chunks

            for h in range(HH):
                hb, hr = h // 2, (h % 2) * 64
                hr2 = 64 - hr
                vcol = h * (HD + 1)
                for qh in range(S // QH):
                    gidx = 2 * h + qh
                    q0 = qh * QH
                    at = patps.tile([65, QH], F32, tag="at",
                                    name=f"at{h}_{qh}")
                    nkb = (q0 + QH) // 128

                    def scores(kb):
                        k0 = kb * 128
                        lo = max(k0 - q0, 0)
                        sc = pscps.tile([128, QH], F32, tag="sc",
                                        name=f"sc{h}_{qh}_{kb}")
                        if kb % 2 == 0:
                            kT, qT, rp = kT_t[hb], qT_t[hb], hr
                        else:
                            kT, qT, rp = kT2_t[hb], qT2_t[hb], hr2
                        for (c0, c1) in chunk_cols(lo):
                            nc.tensor.matmul(
                                sc[:, c0:c1],
                                lhsT=kT[rp:rp + 64, k0:k0 + 128],
                                rhs=qT[rp:rp + 64, q0 + c0:q0 + c1],
                                start=True, stop=True,
                                tile_position=(rp, 0),
                            )
                        return sc

                    def exp(kb, sc):
                        k0 = kb * 128
                        lo = max(k0 - q0, 0)
                        et = pexp.tile([128, QH], DT, tag="exp",
                                       name=f"et{h}_{qh}_{kb}")
                        nc.scalar.activation(et[:, lo:QH], sc[:, lo:QH],
                                             EXP, scale=1.0 / np.sqrt(HD))
                        if k0 >= q0:
                            nc.vector.tensor_mul(et[:, lo:lo + 128],
                                                 et[:, lo:lo + 128],
                                                 mask_t[:])
                        return et

                    def pv(kb, et):
                        lo = max(kb * 128 - q0, 0)
                        for (c0, c1) in chunk_cols(lo):
                            nc.tensor.matmul(
                                at[0:65, c0:c1],
                                lhsT=v_t[kb][:, vcol:vcol + HD + 1],
                                rhs=et[:, c0:c1],
                                start=(kb == 0),
                                stop=(kb == (q0 + c1 - 1) // 128),
                            )

                    def est(kb):
                        lo = max(kb * 128 - q0, 0)
                        n = QH - lo
                        return (n + 352) / 1.2, n / 2.4

                    # Emission order per pair-group g.  The PE queue is
                    # in-order, so anything emitted ahead of ready work
                    # head-of-line-blocks it; the PVs therefore LAG one
                    # group (their exps finished last group), filler comes
                    # next (dependency-free), and the lookahead scores pair
                    # goes last (its sc-slot WAR on this group's exps has
                    # cleared by the time the PE drains the earlier work).
                    #   ACT: exp(2g), exp(2g+1)
                    #   PE : pv(2g-2), pv(2g-1) | filler | scores(2g+2/3)
                    npair = nkb // 2
                    force_filler((gidx, 0))
                    sc_pair = [scores(0), scores(1)]
                    prev_ets = None
                    for g in range(npair):
                        et0 = exp(2 * g, sc_pair[0])
                        et1 = exp(2 * g + 1, sc_pair[1])
                        if prev_ets is not None:
                            pv(2 * g - 2, prev_ets[0])
                            pv(2 * g - 1, prev_ets[1])
                        a0, p0 = est(2 * g)
                        a1, p1 = est(2 * g + 1)
                        # scores pair is concurrent (~p0), PVs are serial
                        debt[0] += (a0 + a1) - (p0 + p0 + p1)
                        emit_filler_by_debt((gidx, g))
                        if g + 1 < npair:
                            force_filler((gidx, g + 1))
                            sc_pair = [scores(2 * g + 2), scores(2 * g + 3)]
                        prev_ets = (et0, et1)
                    pv(nkb - 2, prev_ets[0])
                    pv(nkb - 1, prev_ets[1])

                    # drain + normalize off the critical path.  (The fast
                    # reciprocal needs a partition-0 SBUF input, hence the
                    # dn copy.)
                    au = pau.tile([64, QH], F32, tag="au", name=f"au{h}_{qh}")
                    nc.vector.tensor_copy(au[:], at[0:64, :])
                    dn = prc.tile([1, QH], F32, tag="dn", name=f"dn{h}_{qh}")
                    nc.vector.tensor_copy(dn[:], at[64:65, :])
                    rc = prc.tile([1, QH], F32, tag="rc", name=f"rc{h}_{qh}")
                    nc.vector.reciprocal_approx_fast(rc[:], dn[:])
                    bc = prc.tile([64, QH], F32, tag="bc", name=f"bc{h}_{qh}")
                    nc.gpsimd.partition_broadcast(bc[:], rc[:])
                    nc.gpsimd.tensor_tensor(
                        aT_t[hb][hr:hr + 64, q0:q0 + QH],
                        au[:],
                        bc[:],
                        op=MULT,
                    )

            # ---- phase C: leftover filler + outproj rows 1024-2047 ----
            while filler:
                unit = filler.pop(0)
                unit[3]()
            for sb in range(8, NS):
                for jc in range(2):
                    o_group(sb, jc)

    nc.compile()
    return nc


def _get_nc(dt_mode):
    if dt_mode not in _CACHE:
        _CACHE[dt_mode] = _build(dt_mode)
    return _CACHE[dt_mode]


def make_in_maps(x, Wq_w, Wq_b, Wk_w, Wk_b, Wv_w, Wv_b, Wo_w, Wo_b, np_dt):
    in_maps = []
    for core in range(N_CORES):
        b, half = core // 2, core % 2
        sl = slice(half * DH, (half + 1) * DH)
        in_maps.append({
            "xT": np.ascontiguousarray(x[b].T).astype(np_dt),
            "wq": np.ascontiguousarray(Wq_w[:, sl]).astype(np_dt),
            "wk": np.ascontiguousarray(Wk_w[:, sl]).astype(np_dt),
            "wv": np.ascontiguousarray(Wv_w[:, sl]).astype(np_dt),
            "wo": np.ascontiguousarray(Wo_w[sl, :]).astype(np_dt),
            "bq": np.ascontiguousarray(Wq_b[sl].reshape(-1, 128).T),
            "bk": np.ascontiguousarray(Wk_b[sl].reshape(-1, 128).T),
            "bv": np.broadcast_to(Wv_b[sl], (128, DH)).copy(),
        })
    return in_maps


def kernel(x, Wq_w, Wq_b, Wk_w, Wk_b, Wv_w, Wv_b, Wo_w, Wo_b):
    from concourse.bass_utils import run_bass_kernel_spmd

    np_dt = ml_dtypes.bfloat16 if DT_MODE == "bf16" else np.float32

    args = [np.asarray(a, np.float32) for a in
            (x, Wq_w, Wq_b, Wk_w, Wk_b, Wv_w, Wv_b, Wo_w, Wo_b)]
    x, Wq_w, Wq_b, Wk_w, Wk_b, Wv_w, Wv_b, Wo_w, Wo_b = args

    nc = _get_nc(DT_MODE)
    in_maps = make_in_maps(x, Wq_w, Wq_b, Wk_w, Wk_b, Wv_w, Wv_b, Wo_w, Wo_b,
                           np_dt)
    res = run_bass_kernel_spmd(nc, in_maps, list(range(N_CORES)))

    out = np.empty((B, S, D), np.float32)
    for b in range(B):
        out[b] = (res.results[2 * b]["out"].astype(np.float32)
                  + res.results[2 * b + 1]["out"].astype(np.float32) + Wo_b)
    return out


# revision 19
# speedup vs baseline: 1.0539x; 1.0372x over previous
"""Trainium2 Bass kernel for nn_MultiHeadAttention (B=4, S=2048, D=1024, H=16).

Sharding: 8 cores = batch(4) x head-half(2).  Each core computes, for its
batch element, 8 of the 16 heads: QKV projections against column-sliced
weights, causal attention, and the output projection against the matching
row-slice of Wo.  The two bf16 partial outputs per batch element are summed
on the host (replaces the tensor-parallel all-reduce), and Wo_b is added
there.

v4 design notes:
- Attention scores use the transposed layout scoresT[k, q]; the softmax
  denominator comes from an all-ones column prepended to each head's V
  block (so it lands on psum partition 0, where the fast reciprocal can
  read it straight out of PSUM); normalization runs off the critical path.
- The K=64 scores matmuls are row-packed in the PE array: even kb targets
  array rows 0-63, odd kb rows 64-127 (explicit tile_position), using
  half-swapped copies qT2/kT2 so the operands sit on the matching SBUF
  partitions.  Adjacent even/odd scores matmuls run concurrently -> ~2x
  scores throughput.
- exp on the Scalar engine is the attention-phase serial bottleneck
  (~166us).  The PE stream is kept stall-free (idle gaps re-throttle the
  PE clock to 1.2 GHz): scores for kb-pair g+1 are emitted one group ahead
  of the exps of pair g, and projection / output-projection matmul groups
  are paced into the attention stream by a cycle ledger so the PE always
  has dependency-free work while ACT churns through the exps.
"""

import sys

if "/opt/trn_rl_repo" not in sys.path:
    sys.path.insert(0, "/opt/trn_rl_repo")

import numpy as np
import ml_dtypes

B, S, D = 4, 2048, 1024
H, HD = 16, 64
HH = H // 2          # heads per core
DH = D // 2          # local attention feature dim (HH * HD)
N_CORES = 8
QH = 1024            # q-range processed per attention pass (psum budget)

DT_MODE = "bf16"

_CACHE = {}


def _build(dt_mode):
    import concourse.bass as bass
    import concourse.mybir as mybir
    from concourse import bacc
    from concourse.tile import TileContext
    from concourse.masks import make_upper_triangular

    F32 = mybir.dt.float32
    DT = mybir.dt.bfloat16 if dt_mode == "bf16" else mybir.dt.float32

    ADD = mybir.AluOpType.add
    MULT = mybir.AluOpType.mult
    EXP = mybir.ActivationFunctionType.Exp

    nc = bacc.Bacc("TRN2", target_bir_lowering=False, debug=False,
                   num_devices=N_CORES)

    # x arrives pre-blocked host-side: block (db, cc) of the transposed
    # activation sits at rows (db*NSC+cc)*128, so every DMA is a fully
    # sequential dram read (the [D, S] layout's 1KB lines with 4KB stride
    # throttled the input stream to ~half bandwidth).
    xT = nc.dram_tensor("xT", [D * (S // 512), 512], DT,
                        kind="ExternalInput").ap()
    wq = nc.dram_tensor("wq", [D, DH], DT, kind="ExternalInput").ap()
    wk = nc.dram_tensor("wk", [D, DH], DT, kind="ExternalInput").ap()
    wv = nc.dram_tensor("wv", [D, DH], DT, kind="ExternalInput").ap()
    wo = nc.dram_tensor("wo", [DH, D], DT, kind="ExternalInput").ap()
    bq = nc.dram_tensor("bq", [128, DH // 128], F32, kind="ExternalInput").ap()
    bk = nc.dram_tensor("bk", [128, DH // 128], F32, kind="ExternalInput").ap()
    bv = nc.dram_tensor("bv", [128, DH], F32, kind="ExternalInput").ap()
    # out is likewise written in (sb, jc) blocks and reassembled on host
    out = nc.dram_tensor("out", [S * 2, 512], DT, kind="ExternalOutput").ap()

    ND = D // 128        # 8 contraction tiles over D
    NS = S // 128        # 16 s-blocks
    NJ = DH // 128       # 4 feature tiles of the local 512 dim
    NSC = S // 512       # 4 columns of 512 over S

    from contextlib import ExitStack

    with TileContext(nc) as tc:
        with ExitStack() as stack:
            pool = lambda *a, **kw: stack.enter_context(tc.tile_pool(*a, **kw))
            pp = pool(name="persist", bufs=1)
            pqT = pool(name="qT", bufs=NJ)
            pkT = pool(name="kT", bufs=NJ)
            pqT2 = pool(name="qT2", bufs=NJ)
            pkT2 = pool(name="kT2", bufs=NJ)
            pv = pool(name="vaug", bufs=NS)
            pattnT = pool(name="attnT", bufs=NJ)
            pxt = pool(name="xt", bufs=ND)
            pw = pool(name="w", bufs=3 * ND)
            pwo = pool(name="wo", bufs=NJ)
            pproj = pool(name="projps", bufs=2, space="PSUM")
            pexp = pool(name="exp", bufs=4)
            pau = pool(name="au", bufs=2)
            pof = pool(name="of", bufs=2)
            prc = pool(name="recip", bufs=2)
            pscps = pool(name="scps", bufs=2, space="PSUM")
            patps = pool(name="atps", bufs=1, space="PSUM")
            # ---- constants / biases ----
            bq_t = pp.tile([128, NJ], F32, tag="bq")
            nc.sync.dma_start(bq_t[:], bq[:])
            bk_t = pp.tile([128, NJ], F32, tag="bk")
            nc.sync.dma_start(bk_t[:], bk[:])
            bv_t = pp.tile([128, DH], F32, tag="bv")
            nc.sync.dma_start(bv_t[:], bv[:])
            ones_t = pp.tile([128, HH], F32, tag="ones")
            nc.gpsimd.memset(ones_t[:], 1.0)
            # causal mask for diagonal 128x128 squares of scoresT[k, q]:
            # valid (k <= q) <=> partition p <= free f -> upper-tri incl diag.
            mask_f = pp.tile([128, 128], F32, tag="maskf")
            make_upper_triangular(nc, mask_f[:], val=1.0, diag=True)
            mask_t = pp.tile([128, 128], DT, tag="mask")
            nc.vector.tensor_copy(mask_t[:], mask_f[:])

            # persistent activation buffers
            qT_t = [pqT.tile([128, S], DT, tag="qT", name=f"qT{i}")
                    for i in range(NJ)]
            kT_t = [pkT.tile([128, S], DT, tag="kT", name=f"kT{i}")
                    for i in range(NJ)]
            qT2_t = [pqT2.tile([128, S], DT, tag="qT2", name=f"qT2{i}")
                     for i in range(NJ)]
            kT2_t = [pkT2.tile([128, S], DT, tag="kT2", name=f"kT2{i}")
                     for i in range(NJ)]
            v_t = [pv.tile([128, HH * (HD + 1)], DT, tag="vaug",
                           name=f"vaug{i}") for i in range(NS)]
            aT_t = [pattnT.tile([128, S], DT, tag="attnT", name=f"attnT{i}")
                    for i in range(NJ)]

            # input DMAs, ordered so the first V matmuls start early
            xt_t = [pxt.tile([128, S], DT, tag="xt", name=f"xt{i}")
                    for i in range(ND)]
            wv_t = [pw.tile([128, DH], DT, tag="w3", name=f"wv{db}")
                    for db in range(ND)]
            wq_t = [pw.tile([128, DH], DT, tag="w3", name=f"wq{db}")
                    for db in range(ND)]
            wk_t = [pw.tile([128, DH], DT, tag="w3", name=f"wk{db}")
                    for db in range(ND)]
            wo_t = [pwo.tile([128, D], DT, tag="wo", name=f"wo{db}")
                    for db in range(NJ)]

            def dma_w(w_t, ap):
                for db in range(ND):
                    nc.sync.dma_start(w_t[db][:], ap[db * 128:(db + 1) * 128, :])

            def dma_x(cc):
                cs = slice(cc * 512, (cc + 1) * 512)
                for db in range(ND):
                    r0 = (db * NSC + cc) * 128
                    nc.sync.dma_start(xt_t[db][:, cs], xT[r0:r0 + 128, :])

            dma_w(wv_t, wv)
            dma_x(0)
            dma_x(1)
            dma_w(wq_t, wq)
            dma_w(wk_t, wk)
            dma_x(2)
            dma_x(3)
            for db in range(NJ):
                nc.sync.dma_start(wo_t[db][:], wo[db * 128:(db + 1) * 128, :])

            # ---- HAM warm-up: ~60 dummy matmuls on the first-arriving
            # weight tiles keep the PE busy through the DMA lead-in so the
            # clock gate is already at 8/8 when real work starts.
            warm_ps = pproj.tile([128, 512], F32, tag="proj", name="warm")
            for i in range(24):
                db = i % ND
                nc.tensor.matmul(warm_ps[:], lhsT=wv_t[db][:, 0:128],
                                 rhs=wv_t[db][:], start=True, stop=True)

            # ---- work-unit emitters (each ~850ns of PE work) ----
            live_ps = {}

            def v_half(sb, half):
                name = f"psv{sb}"
                if half == 0:
                    ps = pproj.tile([128, 512], F32, tag="proj", name=name)
                    live_ps[name] = ps
                else:
                    ps = live_ps.pop(name)
                for db in range(4 * half, 4 * half + 4):
                    nc.tensor.matmul(
                        ps[:],
                        lhsT=xt_t[db][:, sb * 128:(sb + 1) * 128],
                        rhs=wv_t[db][:],
                        start=(db == 0), stop=(db == ND - 1),
                    )
                if half == 1:
                    # v_aug layout per head: [v+bias | ones] (65 cols); the
                    # ones column makes psum row 64 the softmax denominator.
                    vt = v_t[sb]
                    v3 = vt[:].rearrange("p (h e) -> p h e", e=HD + 1)
                    nc.vector.tensor_tensor(
                        v3[:, :, 0:HD],
                        ps[:].rearrange("p (h e) -> p h e", e=HD),
                        bv_t[:].rearrange("p (h e) -> p h e", e=HD),
                        op=ADD,
                    )
                    nc.vector.tensor_copy(
                        v3[:, :, HD:HD + 1],
                        ones_t[:].rearrange("p (h e) -> p h e", e=1),
                    )

            def qk_half(w_t, bias_t, dstT, jb, sc, half):
                name = f"ps{'q' if dstT is qT_t else 'k'}{jb}_{sc}"
                if half == 0:
                    ps = pproj.tile([128, 512], F32, tag="proj", name=name)
                    live_ps[name] = ps
                else:
                    ps = live_ps.pop(name)
                for db in range(4 * half, 4 * half + 4):
                    nc.tensor.matmul(
                        ps[:],
                        lhsT=w_t[db][:, jb * 128:(jb + 1) * 128],
                        rhs=xt_t[db][:, sc * 512:(sc + 1) * 512],
                        start=(db == 0), stop=(db == ND - 1),
                    )
                if half == 1:
                    nc.vector.tensor_scalar_add(
                        dstT[jb][:, sc * 512:(sc + 1) * 512],
                        ps[:], bias_t[:, jb:jb + 1],
                    )

            def swap_one(src, dst, c0, c1):
                nc.vector.tensor_copy(dst[0:64, c0:c1], src[64:128, c0:c1])
                nc.vector.tensor_copy(dst[64:128, c0:c1], src[0:64, c0:c1])

            def swap_copy(jb, c0=0, c1=S):
                # half-swapped copies so odd-kb scores matmuls can target
                # the other PE row group
                swap_one(qT_t[jb], qT2_t[jb], c0, c1)
                swap_one(kT_t[jb], kT2_t[jb], c0, c1)

            def o_group(sb, jc):
                ps = pproj.tile([128, 512], F32, tag="proj",
                                name=f"pso{sb}_{jc}")
                for db in range(NJ):
                    nc.tensor.matmul(
                        ps[:],
                        lhsT=aT_t[db][:, sb * 128:(sb + 1) * 128],
                        rhs=wo_t[db][:, jc * 512:(jc + 1) * 512],
                        start=(db == 0), stop=(db == NJ - 1),
                    )
                ot = pof.tile([128, 512], DT, tag="of", name=f"ot{sb}_{jc}")
                nc.vector.tensor_copy(ot[:], ps[:])
                r0 = (sb * 2 + jc) * 128
                nc.sync.dma_start(out[r0:r0 + 128, :], ot[:])

            # filler queue: (req_key, min_key, pe_ns, closure).
            # req_key = (gidx, g) by which the unit MUST have been emitted
            # (dependency order); gidx = 2*h + qh, g = kb-pair index within
            # that (h, qh).  min_key: earliest (gidx, g) at which the unit
            # MAY run (outproj needs all heads' qh=0 rows written AND the
            # normalize chain drained; staggering avoids head-of-line
            # blocking the PE on a late aT write).
            ZERO = (0, 0)
            filler = []
            for sb in range(4, 8):           # v for kb 4-7: jit in h0/qh0
                for half in range(2):
                    filler.append(((0, sb // 2), ZERO, 853,
                                   lambda s=sb, hf=half: v_half(s, hf)))
            for sc in (2, 3):                # qT jb0 cols 1024-2047: h0/qh1
                for half in range(2):
                    filler.append(((1, 0), ZERO, 853,
                                   lambda s=sc, hf=half:
                                      qk_half(wq_t, bq_t, qT_t, 0, s, hf)))
            filler.append(((1, 0), ZERO, 0,
                           lambda: swap_one(qT_t[0], qT2_t[0], 1024, 2048)))
            for sc in (2, 3):                # kT jb0 rows 1024-2047: h0/qh1
                for half in range(2):
                    filler.append(((1, 4), ZERO, 853,
                                   lambda s=sc, hf=half:
                                      qk_half(wk_t, bk_t, kT_t, 0, s, hf)))
            filler.append(((1, 4), ZERO, 0,
                           lambda: swap_one(kT_t[0], kT2_t[0], 1024, 2048)))
            for sb in range(8, NS):          # v for kb 8-15: jit in h0/qh1
                for half in range(2):
                    filler.append(((1, sb // 2), ZERO, 853,
                                   lambda s=sb, hf=half: v_half(s, hf)))
            for jb in range(1, NJ):          # qk for jb: by h=2*jb
                for sc in range(NSC):
                    for w_t, bias_t, dstT in ((wq_t, bq_t, qT_t),
                                              (wk_t, bk_t, kT_t)):
                        for half in range(2):
                            filler.append(
                                ((4 * jb, 0), ZERO, 853,
                                 lambda w=w_t, b=bias_t, d=dstT, j=jb,
                                        s=sc, hf=half:
                                    qk_half(w, b, d, j, s, hf)))
                filler.append(((4 * jb, 0), ZERO, 0,
                               lambda j=jb: swap_copy(j)))
            i = 0                            # outproj rows 0-1023: late h7/qh1
            for sb in range(8):
                for jc in range(2):
                    filler.append(((99, 0), (15, 4 + i // 4), 853,
                                   lambda s=sb, j=jc: o_group(s, j)))
                    i += 1

            debt = [0.0]

            def emit_filler_by_debt(cur_key):
                while debt[0] > 400 and filler and filler[0][1] <= cur_key:
                    unit = filler.pop(0)
                    unit[3]()
                    debt[0] -= unit[2]
                debt[0] = min(debt[0], 3000.0)

            def force_filler(key):
                while filler and filler[0][0] <= key:
                    unit = filler.pop(0)
                    unit[3]()
                    debt[0] -= unit[2]

            # ---- phase A: V(sb 0-3) + QK(jb=0, cols 0-1023); the swaps
            # go right after their producers so they sit early in the DVE
            # queue (h0's first odd-kb scores wait on them) ----
            for sb in range(4):
                v_half(sb, 0)
                v_half(sb, 1)
            for w_t, bias_t, dstT in ((wq_t, bq_t, qT_t),
                                      (wk_t, bk_t, kT_t)):
                for sc in (0, 1):
                    qk_half(w_t, bias_t, dstT, 0, sc, 0)
                    qk_half(w_t, bias_t, dstT, 0, sc, 1)
                if dstT is qT_t:
                    swap_one(qT_t[0], qT2_t[0], 0, 1024)
                else:
                    swap_one(kT_t[0], kT2_t[0], 0, 1024)

            # ---- phase B: attention ----
            def chunk_cols(lo):
                chunks = []
                c = lo
                while c < QH:
                    c1 = min((c // 512 + 1) * 512, QH)
                    chunks.append((c, c1))
                    c = c1
                return chunks

            for h in range(HH):
                hb, hr = h // 2, (h % 2) * 64
                hr2 = 64 - hr
                vcol = h * (HD + 1)
                for qh in range(S // QH):
                    gidx = 2 * h + qh
                    q0 = qh * QH
                    at = patps.tile([65, QH], F32, tag="at",
                                    name=f"at{h}_{qh}")
                    nkb = (q0 + QH) // 128

                    def scores(kb):
                        k0 = kb * 128
                        lo = max(k0 - q0, 0)
                        sc = pscps.tile([128, QH], F32, tag="sc",
                                        name=f"sc{h}_{qh}_{kb}")
                        if kb % 2 == 0:
                            kT, qT, rp = kT_t[hb], qT_t[hb], hr
                        else:
                            kT, qT, rp = kT2_t[hb], qT2_t[hb], hr2
                        for (c0, c1) in chunk_cols(lo):
                            nc.tensor.matmul(
                                sc[:, c0:c1],
                                lhsT=kT[rp:rp + 64, k0:k0 + 128],
                                rhs=qT[rp:rp + 64, q0 + c0:q0 + c1],
                                start=True, stop=True,
                                tile_position=(rp, 0),
                            )
                        return sc

                    def exp(kb, sc):
                        k0 = kb * 128
                        lo = max(k0 - q0, 0)
                        et = pexp.tile([128, QH], DT, tag="exp",
                                       name=f"et{h}_{qh}_{kb}")
                        nc.scalar.activation(et[:, lo:QH], sc[:, lo:QH],
                                             EXP, scale=1.0 / np.sqrt(HD))
                        if k0 >= q0:
                            nc.vector.tensor_mul(et[:, lo:lo + 128],
                                                 et[:, lo:lo + 128],
                                                 mask_t[:])
                        return et

                    def pv(kb, et):
                        lo = max(kb * 128 - q0, 0)
                        for (c0, c1) in chunk_cols(lo):
                            nc.tensor.matmul(
                                at[0:65, c0:c1],
                                lhsT=v_t[kb][:, vcol:vcol + HD + 1],
                                rhs=et[:, c0:c1],
                                start=(kb == 0),
                                stop=(kb == (q0 + c1 - 1) // 128),
                            )

                    def est(kb):
                        lo = max(kb * 128 - q0, 0)
                        n = QH - lo
                        return (n + 352) / 1.2, n / 2.4

                    # Emission order per pair-group g.  The PE queue is
                    # in-order, so anything emitted ahead of ready work
                    # head-of-line-blocks it; the PVs therefore LAG one
                    # group (their exps finished last group), filler comes
                    # next (dependency-free), and the lookahead scores pair
                    # goes last (its sc-slot WAR on this group's exps has
                    # cleared by the time the PE drains the earlier work).
                    #   ACT: exp(2g), exp(2g+1)
                    #   PE : pv(2g-2), pv(2g-1) | filler | scores(2g+2/3)
                    npair = nkb // 2
                    force_filler((gidx, 0))
                    sc_pair = [scores(0), scores(1)]
                    prev_ets = None
                    for g in range(npair):
                        et0 = exp(2 * g, sc_pair[0])
                        et1 = exp(2 * g + 1, sc_pair[1])
                        if prev_ets is not None:
                            pv(2 * g - 2, prev_ets[0])
                            pv(2 * g - 1, prev_ets[1])
                        a0, p0 = est(2 * g)
                        a1, p1 = est(2 * g + 1)
                        # scores pair is concurrent (~p0), PVs are serial
                        debt[0] += (a0 + a1) - (p0 + p0 + p1)
                        emit_filler_by_debt((gidx, g))
                        if g + 1 < npair:
                            force_filler((gidx, g + 1))
                            sc_pair = [scores(2 * g + 2), scores(2 * g + 3)]
                        prev_ets = (et0, et1)
                    pv(nkb - 2, prev_ets[0])
                    pv(nkb - 1, prev_ets[1])

                    # drain + normalize off the critical path.  (The fast
                    # reciprocal needs a partition-0 SBUF input, hence the
                    # dn copy.)
                    au = pau.tile([64, QH], F32, tag="au", name=f"au{h}_{qh}")
                    nc.vector.tensor_copy(au[:], at[0:64, :])
                    dn = prc.tile([1, QH], F32, tag="dn", name=f"dn{h}_{qh}")
                    nc.vector.tensor_copy(dn[:], at[64:65, :])
                    rc = prc.tile([1, QH], F32, tag="rc", name=f"rc{h}_{qh}")
                    nc.vector.reciprocal_approx_fast(rc[:], dn[:])
                    bc = prc.tile([64, QH], F32, tag="bc", name=f"bc{h}_{qh}")
                    nc.gpsimd.partition_broadcast(bc[:], rc[:])
                    nc.gpsimd.tensor_tensor(
                        aT_t[hb][hr:hr + 64, q0:q0 + QH],
                        au[:],
                        bc[:],
                        op=MULT,
                    )

            # ---- phase C: leftover filler + outproj rows 1024-2047 ----
            while filler:
                unit = filler.pop(0)
                unit[3]()
            for sb in range(8, NS):
                for jc in range(2):
                    o_group(sb, jc)

    nc.compile()
    return nc


def _get_nc(dt_mode):
    if dt_mode not in _CACHE:
        _CACHE[dt_mode] = _build(dt_mode)
    return _CACHE[dt_mode]


def make_in_maps(x, Wq_w, Wq_b, Wk_w, Wk_b, Wv_w, Wv_b, Wo_w, Wo_b, np_dt):
    in_maps = []
    for core in range(N_CORES):
        b, half = core // 2, core % 2
        sl = slice(half * DH, (half + 1) * DH)
        in_maps.append({
            "xT": np.ascontiguousarray(
                x[b].T.reshape(8, 128, 4, 512).transpose(0, 2, 1, 3)
                .reshape(-1, 512)).astype(np_dt),
            "wq": np.ascontiguousarray(Wq_w[:, sl]).astype(np_dt),
            "wk": np.ascontiguousarray(Wk_w[:, sl]).astype(np_dt),
            "wv": np.ascontiguousarray(Wv_w[:, sl]).astype(np_dt),
            "wo": np.ascontiguousarray(Wo_w[sl, :]).astype(np_dt),
            "bq": np.ascontiguousarray(Wq_b[sl].reshape(-1, 128).T),
            "bk": np.ascontiguousarray(Wk_b[sl].reshape(-1, 128).T),
            "bv": np.broadcast_to(Wv_b[sl], (128, DH)).copy(),
        })
    return in_maps


def kernel(x, Wq_w, Wq_b, Wk_w, Wk_b, Wv_w, Wv_b, Wo_w, Wo_b):
    from concourse.bass_utils import run_bass_kernel_spmd

    np_dt = ml_dtypes.bfloat16 if DT_MODE == "bf16" else np.float32

    args = [np.asarray(a, np.float32) for a in
            (x, Wq_w, Wq_b, Wk_w, Wk_b, Wv_w, Wv_b, Wo_w, Wo_b)]
    x, Wq_w, Wq_b, Wk_w, Wk_b, Wv_w, Wv_b, Wo_w, Wo_b = args

    nc = _get_nc(DT_MODE)
    in_maps = make_in_maps(x, Wq_w, Wq_b, Wk_w, Wk_b, Wv_w, Wv_b, Wo_w, Wo_b,
                           np_dt)
    res = run_bass_kernel_spmd(nc, in_maps, list(range(N_CORES)))

    def unblock(o):
        # [16*2*128, 512] blocks -> [S, D]
        return o.reshape(16, 2, 128, 512).transpose(0, 2, 1, 3).reshape(S, D)

    out = np.empty((B, S, D), np.float32)
    for b in range(B):
        out[b] = (unblock(res.results[2 * b]["out"].astype(np.float32))
                  + unblock(res.results[2 * b + 1]["out"].astype(np.float32))
                  + Wo_b)
    return out


# revision 26
# speedup vs baseline: 1.1916x; 1.1306x over previous
"""Trainium2 Bass kernel for nn_MultiHeadAttention (B=4, S=2048, D=1024, H=16).

Sharding: 8 cores = batch(4) x head-half(2).  Each core computes, for its
batch element, 8 of the 16 heads: QKV projections against column-sliced
weights, causal attention, and the output projection against the matching
row-slice of Wo.  The two bf16 partial outputs per batch element are summed
on the host (replaces the tensor-parallel all-reduce), and Wo_b is added
there.

v4 design notes:
- Attention scores use the transposed layout scoresT[k, q]; the softmax
  denominator comes from an all-ones column prepended to each head's V
  block (so it lands on psum partition 0, where the fast reciprocal can
  read it straight out of PSUM); normalization runs off the critical path.
- The K=64 scores matmuls are row-packed in the PE array: even kb targets
  array rows 0-63, odd kb rows 64-127 (explicit tile_position), using
  half-swapped copies qT2/kT2 so the operands sit on the matching SBUF
  partitions.  Adjacent even/odd scores matmuls run concurrently -> ~2x
  scores throughput.
- exp on the Scalar engine is the attention-phase serial bottleneck
  (~166us).  The PE stream is kept stall-free (idle gaps re-throttle the
  PE clock to 1.2 GHz): scores for kb-pair g+1 are emitted one group ahead
  of the exps of pair g, and projection / output-projection matmul groups
  are paced into the attention stream by a cycle ledger so the PE always
  has dependency-free work while ACT churns through the exps.
"""

import sys

if "/opt/trn_rl_repo" not in sys.path:
    sys.path.insert(0, "/opt/trn_rl_repo")

import numpy as np
import ml_dtypes

B, S, D = 4, 2048, 1024
H, HD = 16, 64
HH = H // 2          # heads per core
DH = D // 2          # local attention feature dim (HH * HD)
N_CORES = 8
QH = 1024            # q-range processed per attention pass (psum budget)

DT_MODE = "bf16"

_CACHE = {}


def _build(dt_mode):
    import concourse.bass as bass
    import concourse.mybir as mybir
    from concourse import bacc
    from concourse.tile import TileContext
    from concourse.masks import make_upper_triangular

    F32 = mybir.dt.float32
    DT = mybir.dt.bfloat16 if dt_mode == "bf16" else mybir.dt.float32

    ADD = mybir.AluOpType.add
    MULT = mybir.AluOpType.mult
    EXP = mybir.ActivationFunctionType.Exp

    nc = bacc.Bacc("TRN2", target_bir_lowering=False, debug=False,
                   num_devices=N_CORES)

    # x arrives pre-blocked host-side: block (db, cc) of the transposed
    # activation sits at rows (db*NSC+cc)*128, so every DMA is a fully
    # sequential dram read (the [D, S] layout's 1KB lines with 4KB stride
    # throttled the input stream to ~half bandwidth).
    xT = nc.dram_tensor("xT", [D * (S // 512), 512], DT,
                        kind="ExternalInput").ap()
    wq = nc.dram_tensor("wq", [D, DH], DT, kind="ExternalInput").ap()
    wk = nc.dram_tensor("wk", [D, DH], DT, kind="ExternalInput").ap()
    wv = nc.dram_tensor("wv", [D, DH], DT, kind="ExternalInput").ap()
    wo = nc.dram_tensor("wo", [DH, D], DT, kind="ExternalInput").ap()
    bq = nc.dram_tensor("bq", [128, DH // 128], F32, kind="ExternalInput").ap()
    bk = nc.dram_tensor("bk", [128, DH // 128], F32, kind="ExternalInput").ap()
    bv = nc.dram_tensor("bv", [128, DH], F32, kind="ExternalInput").ap()
    # out is likewise written in (sb, jc) blocks and reassembled on host
    out = nc.dram_tensor("out", [S * 2, 512], DT, kind="ExternalOutput").ap()

    ND = D // 128        # 8 contraction tiles over D
    NS = S // 128        # 16 s-blocks
    NJ = DH // 128       # 4 feature tiles of the local 512 dim
    NSC = S // 512       # 4 columns of 512 over S

    from contextlib import ExitStack

    with TileContext(nc) as tc:
        with ExitStack() as stack:
            pool = lambda *a, **kw: stack.enter_context(tc.tile_pool(*a, **kw))
            pp = pool(name="persist", bufs=1)
            pqT = pool(name="qT", bufs=NJ)
            pkT = pool(name="kT", bufs=NJ)
            pqT2 = pool(name="qT2", bufs=NJ)
            pkT2 = pool(name="kT2", bufs=NJ)
            pv = pool(name="vaug", bufs=NS)
            pattnT = pool(name="attnT", bufs=NJ)
            pxt = pool(name="xt", bufs=ND)
            pw = pool(name="w", bufs=3 * ND)
            pwo = pool(name="wo", bufs=NJ)
            pproj = pool(name="projps", bufs=2, space="PSUM")
            pexp = pool(name="exp", bufs=4)
            pau = pool(name="au", bufs=2)
            pof = pool(name="of", bufs=2)
            prc = pool(name="recip", bufs=2)
            pscps = pool(name="scps", bufs=2, space="PSUM")
            patps = pool(name="atps", bufs=1, space="PSUM")
            # ---- constants / biases ----
            bq_t = pp.tile([128, NJ], F32, tag="bq")
            nc.sync.dma_start(bq_t[:], bq[:])
            bk_t = pp.tile([128, NJ], F32, tag="bk")
            nc.sync.dma_start(bk_t[:], bk[:])
            bv_t = pp.tile([128, DH], F32, tag="bv")
            nc.sync.dma_start(bv_t[:], bv[:])
            ones64 = pp.tile([128, 64], DT, tag="ones64")
            nc.gpsimd.memset(ones64[:], 1.0)
            # causal mask for diagonal 128x128 squares of scoresT[k, q]:
            # valid (k <= q) <=> partition p <= free f -> upper-tri incl diag.
            mask_f = pp.tile([128, 128], F32, tag="maskf")
            make_upper_triangular(nc, mask_f[:], val=1.0, diag=True)
            mask_t = pp.tile([128, 128], DT, tag="mask")
            nc.vector.tensor_copy(mask_t[:], mask_f[:])

            # persistent activation buffers
            qT_t = [pqT.tile([128, S], DT, tag="qT", name=f"qT{i}")
                    for i in range(NJ)]
            kT_t = [pkT.tile([128, S], DT, tag="kT", name=f"kT{i}")
                    for i in range(NJ)]
            qT2_t = [pqT2.tile([128, S], DT, tag="qT2", name=f"qT2{i}")
                     for i in range(NJ)]
            kT2_t = [pkT2.tile([128, S], DT, tag="kT2", name=f"kT2{i}")
                     for i in range(NJ)]
            v_t = [pv.tile([128, DH], DT, tag="vaug",
                           name=f"vaug{i}") for i in range(NS)]
            aT_t = [pattnT.tile([128, S], DT, tag="attnT", name=f"attnT{i}")
                    for i in range(NJ)]

            # input DMAs, ordered so the first V matmuls start early
            xt_t = [pxt.tile([128, S], DT, tag="xt", name=f"xt{i}")
                    for i in range(ND)]
            wv_t = [pw.tile([128, DH], DT, tag="w3", name=f"wv{db}")
                    for db in range(ND)]
            wq_t = [pw.tile([128, DH], DT, tag="w3", name=f"wq{db}")
                    for db in range(ND)]
            wk_t = [pw.tile([128, DH], DT, tag="w3", name=f"wk{db}")
                    for db in range(ND)]
            wo_t = [pwo.tile([128, D], DT, tag="wo", name=f"wo{db}")
                    for db in range(NJ)]

            def dma_w(w_t, ap):
                for db in range(ND):
                    nc.sync.dma_start(w_t[db][:], ap[db * 128:(db + 1) * 128, :])

            def dma_x(cc):
                cs = slice(cc * 512, (cc + 1) * 512)
                for db in range(ND):
                    r0 = (db * NSC + cc) * 128
                    nc.sync.dma_start(xt_t[db][:, cs], xT[r0:r0 + 128, :])

            dma_w(wv_t, wv)
            dma_x(0)
            dma_x(1)
            dma_w(wq_t, wq)
            dma_w(wk_t, wk)
            dma_x(2)
            dma_x(3)
            for db in range(NJ):
                nc.sync.dma_start(wo_t[db][:], wo[db * 128:(db + 1) * 128, :])

            # ---- HAM warm-up: ~60 dummy matmuls on the first-arriving
            # weight tiles keep the PE busy through the DMA lead-in so the
            # clock gate is already at 8/8 when real work starts.
            warm_ps = pproj.tile([128, 512], F32, tag="proj", name="warm")
            for i in range(24):
                db = i % ND
                nc.tensor.matmul(warm_ps[:], lhsT=wv_t[db][:, 0:128],
                                 rhs=wv_t[db][:], start=True, stop=True)

            # ---- work-unit emitters (each ~850ns of PE work) ----
            live_ps = {}

            def v_half(sb, half):
                name = f"psv{sb}"
                if half == 0:
                    ps = pproj.tile([128, 512], F32, tag="proj", name=name)
                    live_ps[name] = ps
                else:
                    ps = live_ps.pop(name)
                for db in range(4 * half, 4 * half + 4):
                    nc.tensor.matmul(
                        ps[:],
                        lhsT=xt_t[db][:, sb * 128:(sb + 1) * 128],
                        rhs=wv_t[db][:],
                        start=(db == 0), stop=(db == ND - 1),
                    )
                if half == 1:
                    nc.vector.tensor_tensor(v_t[sb][:], ps[:], bv_t[:],
                                            op=ADD)

            def qk_half(w_t, bias_t, dstT, jb, sc, half):
                name = f"ps{'q' if dstT is qT_t else 'k'}{jb}_{sc}"
                if half == 0:
                    ps = pproj.tile([128, 512], F32, tag="proj", name=name)
                    live_ps[name] = ps
                else:
                    ps = live_ps.pop(name)
                for db in range(4 * half, 4 * half + 4):
                    nc.tensor.matmul(
                        ps[:],
                        lhsT=w_t[db][:, jb * 128:(jb + 1) * 128],
                        rhs=xt_t[db][:, sc * 512:(sc + 1) * 512],
                        start=(db == 0), stop=(db == ND - 1),
                    )
                if half == 1:
                    nc.vector.tensor_scalar_add(
                        dstT[jb][:, sc * 512:(sc + 1) * 512],
                        ps[:], bias_t[:, jb:jb + 1],
                    )

            def swap_one(src, dst, c0, c1):
                nc.vector.tensor_copy(dst[0:64, c0:c1], src[64:128, c0:c1])
                nc.vector.tensor_copy(dst[64:128, c0:c1], src[0:64, c0:c1])

            def swap_copy(jb, c0=0, c1=S):
                # half-swapped copies so odd-kb scores matmuls can target
                # the other PE row group
                swap_one(qT_t[jb], qT2_t[jb], c0, c1)
                swap_one(kT_t[jb], kT2_t[jb], c0, c1)

            def o_group(sb, jc):
                ps = pproj.tile([128, 512], F32, tag="proj",
                                name=f"pso{sb}_{jc}")
                for db in range(NJ):
                    nc.tensor.matmul(
                        ps[:],
                        lhsT=aT_t[db][:, sb * 128:(sb + 1) * 128],
                        rhs=wo_t[db][:, jc * 512:(jc + 1) * 512],
                        start=(db == 0), stop=(db == NJ - 1),
                    )
                ot = pof.tile([128, 512], DT, tag="of", name=f"ot{sb}_{jc}")
                nc.vector.tensor_copy(ot[:], ps[:])
                r0 = (sb * 2 + jc) * 128
                nc.sync.dma_start(out[r0:r0 + 128, :], ot[:])

            # filler queue: (req_key, min_key, pe_ns, closure).
            # req_key = (gidx, g) by which the unit MUST have been emitted
            # (dependency order); gidx = 2*h + qh, g = kb-pair index within
            # that (h, qh).  min_key: earliest (gidx, g) at which the unit
            # MAY run (outproj needs all heads' qh=0 rows written AND the
            # normalize chain drained; staggering avoids head-of-line
            # blocking the PE on a late aT write).
            ZERO = (0, 0)
            filler = []
            for sb in range(4, 8):           # v for kb 4-7: jit in h0/qh0
                for half in range(2):
                    filler.append(((0, sb // 2), ZERO, 853,
                                   lambda s=sb, hf=half: v_half(s, hf)))
            for sc in (2, 3):                # qT jb0 cols 1024-2047: h0/qh1
                for half in range(2):
                    filler.append(((1, 0), ZERO, 853,
                                   lambda s=sc, hf=half:
                                      qk_half(wq_t, bq_t, qT_t, 0, s, hf)))
            filler.append(((1, 0), ZERO, 0,
                           lambda: swap_one(qT_t[0], qT2_t[0], 1024, 2048)))
            for sc in (2, 3):                # kT jb0 rows 1024-2047: h0/qh1
                for half in range(2):
                    filler.append(((1, 4), ZERO, 853,
                                   lambda s=sc, hf=half:
                                      qk_half(wk_t, bk_t, kT_t, 0, s, hf)))
            filler.append(((1, 4), ZERO, 0,
                           lambda: swap_one(kT_t[0], kT2_t[0], 1024, 2048)))
            for sb in range(8, NS):          # v for kb 8-15: jit in h0/qh1
                for half in range(2):
                    filler.append(((1, sb // 2), ZERO, 853,
                                   lambda s=sb, hf=half: v_half(s, hf)))
            for jb in range(1, NJ):          # qk for jb: by h=2*jb
                for sc in range(NSC):
                    for w_t, bias_t, dstT in ((wq_t, bq_t, qT_t),
                                              (wk_t, bk_t, kT_t)):
                        for half in range(2):
                            filler.append(
                                ((4 * jb, 0), ZERO, 853,
                                 lambda w=w_t, b=bias_t, d=dstT, j=jb,
                                        s=sc, hf=half:
                                    qk_half(w, b, d, j, s, hf)))
                filler.append(((4 * jb, 0), ZERO, 0,
                               lambda j=jb: swap_copy(j)))
            i = 0                            # outproj rows 0-1023: late h7/qh1
            for sb in range(8):
                for jc in range(2):
                    filler.append(((99, 0), (15, 4 + i // 4), 853,
                                   lambda s=sb, j=jc: o_group(s, j)))
                    i += 1

            debt = [0.0]

            def emit_filler_by_debt(cur_key):
                while debt[0] > 400 and filler and filler[0][1] <= cur_key:
                    unit = filler.pop(0)
                    unit[3]()
                    debt[0] -= unit[2]
                debt[0] = min(debt[0], 3000.0)

            def force_filler(key):
                while filler and filler[0][0] <= key:
                    unit = filler.pop(0)
                    unit[3]()
                    debt[0] -= unit[2]

            # ---- phase A: V(sb 0-3) + QK(jb=0, cols 0-1023); the swaps
            # go right after their producers so they sit early in the DVE
            # queue (h0's first odd-kb scores wait on them) ----
            for sb in range(4):
                v_half(sb, 0)
                v_half(sb, 1)
            for w_t, bias_t, dstT in ((wq_t, bq_t, qT_t),
                                      (wk_t, bk_t, kT_t)):
                for sc in (0, 1):
                    qk_half(w_t, bias_t, dstT, 0, sc, 0)
                    qk_half(w_t, bias_t, dstT, 0, sc, 1)
                if dstT is qT_t:
                    swap_one(qT_t[0], qT2_t[0], 0, 1024)
                else:
                    swap_one(kT_t[0], kT2_t[0], 0, 1024)

            # ---- phase B: attention ----
            def chunk_cols(lo):
                chunks = []
                c = lo
                while c < QH:
                    c1 = min((c // 512 + 1) * 512, QH)
                    chunks.append((c, c1))
                    c = c1
                return chunks

            for h in range(HH):
                hb, hr = h // 2, (h % 2) * 64
                hr2 = 64 - hr
                vcol = h * HD
                for qh in range(S // QH):
                    gidx = 2 * h + qh
                    q0 = qh * QH
                    at = patps.tile([128, QH], F32, tag="at",
                                    name=f"at{h}_{qh}")
                    nkb = (q0 + QH) // 128

                    def scores(kb):
                        k0 = kb * 128
                        lo = max(k0 - q0, 0)
                        sc = pscps.tile([128, QH], F32, tag="sc",
                                        name=f"sc{h}_{qh}_{kb}")
                        if kb % 2 == 0:
                            kT, qT, rp = kT_t[hb], qT_t[hb], hr
                        else:
                            kT, qT, rp = kT2_t[hb], qT2_t[hb], hr2
                        for (c0, c1) in chunk_cols(lo):
                            nc.tensor.matmul(
                                sc[:, c0:c1],
                                lhsT=kT[rp:rp + 64, k0:k0 + 128],
                                rhs=qT[rp:rp + 64, q0 + c0:q0 + c1],
                                start=True, stop=True,
                                tile_position=(rp, 0),
                            )
                        return sc

                    def exp(kb, sc):
                        k0 = kb * 128
                        lo = max(k0 - q0, 0)
                        et = pexp.tile([128, QH], DT, tag="exp",
                                       name=f"et{h}_{qh}_{kb}")
                        nc.scalar.activation(et[:, lo:QH], sc[:, lo:QH],
                                             EXP, scale=1.0 / np.sqrt(HD))
                        if k0 >= q0:
                            nc.vector.tensor_mul(et[:, lo:lo + 128],
                                                 et[:, lo:lo + 128],
                                                 mask_t[:])
                        return et

                    def pv(kb, et):
                        # attention (PE cols 0-63) and an all-ones matmul
                        # (cols 64-127) run concurrently: at[64+j, q] gets
                        # the softmax denominator replicated across all 64
                        # partitions, so the normalize is pure elementwise
                        # DVE work (no gpsimd broadcast on the critical
                        # path).  Only the first matmul carries start=True:
                        # its flags-clear covers the whole bank.
                        lo = max(kb * 128 - q0, 0)
                        for (c0, c1) in chunk_cols(lo):
                            st = (kb == 0)
                            sp = (kb == (q0 + c1 - 1) // 128)
                            nc.tensor.matmul(
                                at[0:64, c0:c1],
                                lhsT=v_t[kb][:, vcol:vcol + HD],
                                rhs=et[:, c0:c1],
                                start=st, stop=sp,
                            )
                            nc.tensor.matmul(
                                at[64:128, c0:c1],
                                lhsT=ones64[:],
                                rhs=et[:, c0:c1],
                                start=st, stop=sp,
                            )

                    def est(kb):
                        lo = max(kb * 128 - q0, 0)
                        n = QH - lo
                        return (n + 352) / 1.2, n / 2.4

                    # Emission order per pair-group g.  The PE queue is
                    # in-order, so anything emitted ahead of ready work
                    # head-of-line-blocks it; the PVs therefore LAG one
                    # group (their exps finished last group), filler comes
                    # next (dependency-free), and the lookahead scores pair
                    # goes last (its sc-slot WAR on this group's exps has
                    # cleared by the time the PE drains the earlier work).
                    #   ACT: exp(2g), exp(2g+1)
                    #   PE : pv(2g-2), pv(2g-1) | filler | scores(2g+2/3)
                    npair = nkb // 2
                    force_filler((gidx, 0))
                    sc_pair = [scores(0), scores(1)]
                    prev_ets = None
                    for g in range(npair):
                        et0 = exp(2 * g, sc_pair[0])
                        et1 = exp(2 * g + 1, sc_pair[1])
                        if prev_ets is not None:
                            pv(2 * g - 2, prev_ets[0])
                            pv(2 * g - 1, prev_ets[1])
                        a0, p0 = est(2 * g)
                        a1, p1 = est(2 * g + 1)
                        # scores pair is concurrent (~p0), PVs are serial
                        debt[0] += (a0 + a1) - (p0 + p0 + p1)
                        emit_filler_by_debt((gidx, g))
                        if g + 1 < npair:
                            force_filler((gidx, g + 1))
                            sc_pair = [scores(2 * g + 2), scores(2 * g + 3)]
                        prev_ets = (et0, et1)
                    pv(nkb - 2, prev_ets[0])
                    pv(nkb - 1, prev_ets[1])

                    # normalize: pure DVE with short latency, so aT lands
                    # promptly and the output projection never waits long
                    dn64 = pau.tile([64, QH], F32, tag="au",
                                    name=f"dn{h}_{qh}")
                    nc.vector.tensor_copy(dn64[:], at[64:128, :])
                    rc64 = prc.tile([64, QH], F32, tag="rc",
                                    name=f"rc{h}_{qh}")
                    nc.vector.reciprocal_approx_fast(rc64[:], dn64[:])
                    nc.vector.tensor_tensor(
                        aT_t[hb][hr:hr + 64, q0:q0 + QH],
                        at[0:64, :],
                        rc64[:],
                        op=MULT,
                    )

            # ---- phase C: leftover filler + outproj rows 1024-2047 ----
            while filler:
                unit = filler.pop(0)
                unit[3]()
            for sb in range(8, NS):
                for jc in range(2):
                    o_group(sb, jc)

    nc.compile()
    return nc


def _get_nc(dt_mode):
    if dt_mode not in _CACHE:
        _CACHE[dt_mode] = _build(dt_mode)
    return _CACHE[dt_mode]


def make_in_maps(x, Wq_w, Wq_b, Wk_w, Wk_b, Wv_w, Wv_b, Wo_w, Wo_b, np_dt):
    in_maps = []
    for core in range(N_CORES):
        b, half = core // 2, core % 2
        sl = slice(half * DH, (half + 1) * DH)
        in_maps.append({
            "xT": np.ascontiguousarray(
                x[b].T.reshape(8, 128, 4, 512).transpose(0, 2, 1, 3)
                .reshape(-1, 512)).astype(np_dt),
            "wq": np.ascontiguousarray(Wq_w[:, sl]).astype(np_dt),
            "wk": np.ascontiguousarray(Wk_w[:, sl]).astype(np_dt),
            "wv": np.ascontiguousarray(Wv_w[:, sl]).astype(np_dt),
            "wo": np.ascontiguousarray(Wo_w[sl, :]).astype(np_dt),
            "bq": np.ascontiguousarray(Wq_b[sl].reshape(-1, 128).T),
            "bk": np.ascontiguousarray(Wk_b[sl].reshape(-1, 128).T),
            "bv": np.broadcast_to(Wv_b[sl], (128, DH)).copy(),
        })
    return in_maps


def kernel(x, Wq_w, Wq_b, Wk_w, Wk_b, Wv_w, Wv_b, Wo_w, Wo_b):
    from concourse.bass_utils import run_bass_kernel_spmd

    np_dt = ml_dtypes.bfloat16 if DT_MODE == "bf16" else np.float32

    args = [np.asarray(a, np.float32) for a in
            (x, Wq_w, Wq_b, Wk_w, Wk_b, Wv_w, Wv_b, Wo_w, Wo_b)]
    x, Wq_w, Wq_b, Wk_w, Wk_b, Wv_w, Wv_b, Wo_w, Wo_b = args

    nc = _get_nc(DT_MODE)
    in_maps = make_in_maps(x, Wq_w, Wq_b, Wk_w, Wk_b, Wv_w, Wv_b, Wo_w, Wo_b,
                           np_dt)
    res = run_bass_kernel_spmd(nc, in_maps, list(range(N_CORES)))

    def unblock(o):
        # [16*2*128, 512] blocks -> [S, D]
        return o.reshape(16, 2, 128, 512).transpose(0, 2, 1, 3).reshape(S, D)

    out = np.empty((B, S, D), np.float32)
    for b in range(B):
        out[b] = (unblock(res.results[2 * b]["out"].astype(np.float32))
                  + unblock(res.results[2 * b + 1]["out"].astype(np.float32))
                  + Wo_b)
    return out
